# revision 1
# baseline (speedup 1.0000x reference)
"""Multi-head attention (B=4, S=2048, D=1024, H=16) on 8 trn2 NeuronCores.

Sharding: (batch, head-half) -> one core each. Core c handles batch c//2 and
heads (c%2)*8 .. (c%2)*8+7 (feature columns (c%2)*512 .. +512 of the QKV
projections, rows of Wo). Each core computes its 8 heads' attention and a
partial output projection; the host sums the two partials per batch and adds
the output bias.

Device layout per core (S=2048 tokens, F=512 local features, hd=64):
  - inputs Q/K/V arrive host-transposed as [1024, 2048] so the d_model
    contraction sits on SBUF partitions,
  - q^T/k^T are produced feature-major ([512, 2048]) via lhsT=W, rhs=X^T,
  - scores are computed transposed (S^T[k, q]) so the P@V matmul can use v
    in natural [token, feature] layout as the stationary operand,
  - softmax: exp on the ACT engine with the 1/8 scale folded in; the
    denominator comes from an all-ones 65th column appended to v; the
    normalization multiplies o'^T rows by a PE-broadcast reciprocal row.
Matmuls run in bf16 (fp32 PSUM accumulation); inputs/weights are cast to
bf16 on the host. Softmax denominators and reciprocals stay fp32. Measured
relative error vs the fp32 reference: ~8.5e-3; HW exec ~540us/core.
"""
import numpy as np

import concourse.bass as bass
import concourse.tile as tile
from concourse import mybir
from concourse.bass_utils import run_bass_kernel_spmd

F32 = mybir.dt.float32
F32R = mybir.dt.float32r
BF16 = mybir.dt.bfloat16
EXP = mybir.ActivationFunctionType.Exp

B, S, DM, H_TOT = 4, 2048, 1024, 16
F = 512          # features per core (8 heads x 64)
HD = 64          # head dim
NH = 8           # heads per core
NP = 4           # head pairs per core
KT = 16          # k tiles of 128
NQT = 4          # q chunks of 512
SCALE = 0.125    # 1/sqrt(64)
N_CORES = 8

_WAIT_CAP = {"InstEventSemaphore": 2}


def _split_multiwaits(nc):
    """This walrus build accepts 1 sync-wait per instruction (2 on
    EventSemaphore); spread extras over same-engine NOPs placed before."""
    n_spill = 0
    for f in nc.m.functions:
        for bb in f.blocks:
            new = []
            changed = False
            for inst in bb.instructions:
                si = inst.sync_info
                cap = _WAIT_CAP.get(type(inst).__name__, 1)
                if si is not None and len(si.on_wait) > cap:
                    extra = list(si.on_wait[: len(si.on_wait) - cap])
                    del si.on_wait[: len(si.on_wait) - cap]
                    for w in extra:
                        n_spill += 1
                        nop = mybir.InstNoOp(name=f"I-wspill-{n_spill}-{inst.name}")
                        nop.engine = inst.engine
                        nop.sync_info = mybir.SyncInfo(on_wait=[w], on_update=[])
                        new.append(nop)
                    changed = True
                new.append(inst)
            if changed:
                bb.instructions[:] = new
    return n_spill


def build_program():
    nc = bass.Bass("TRN2", target_bir_lowering=False, debug=False, num_devices=1)

    d_qt = nc.dram_tensor("qt", [DM, S], BF16, kind="ExternalInput").ap()
    d_kt = nc.dram_tensor("kt", [DM, S], BF16, kind="ExternalInput").ap()
    d_vt = nc.dram_tensor("vt", [DM, S], BF16, kind="ExternalInput").ap()
    d_wq = nc.dram_tensor("wq", [DM, F], BF16, kind="ExternalInput").ap()
    d_wk = nc.dram_tensor("wk", [DM, F], BF16, kind="ExternalInput").ap()
    d_wv = nc.dram_tensor("wv", [DM, F], BF16, kind="ExternalInput").ap()
    d_wo = nc.dram_tensor("wo", [F, DM], BF16, kind="ExternalInput").ap()
    d_bq = nc.dram_tensor("bq", [F], F32, kind="ExternalInput").ap()
    d_bk = nc.dram_tensor("bk", [F], F32, kind="ExternalInput").ap()
    d_bv = nc.dram_tensor("bv", [F], F32R, kind="ExternalInput").ap()
    d_ones = nc.dram_tensor("ones", [1, 128], F32R, kind="ExternalInput").ap()
    d_part = nc.dram_tensor("part", [S, DM], F32, kind="ExternalOutput").ap()

    with tile.TileContext(nc) as tc:
        with (
            tc.tile_pool(name="wpool", bufs=1) as wpool,
            tc.tile_pool(name="big", bufs=1) as big,
            tc.tile_pool(name="inch", bufs=20) as inch,
            tc.tile_pool(name="vtch", bufs=24) as vtch,
            tc.tile_pool(name="exch", bufs=8) as exch,
            tc.tile_pool(name="small", bufs=4) as small,
            tc.tile_pool(name="outst", bufs=2) as outst,
            tc.tile_pool(name="rcp", bufs=2) as rcp,
            tc.tile_pool(name="ocp", bufs=4) as ocp,
            tc.tile_pool(name="ps_sc", bufs=2, space="PSUM") as ps_sc,
            tc.tile_pool(name="ps_pv", bufs=3, space="PSUM") as ps_pv,
            tc.tile_pool(name="ps_misc", bufs=1, space="PSUM") as ps_misc,
        ):
            # ---- resident tiles
            wq_sb = [wpool.tile([128, F], BF16, tag=f"wq{m}", name=f"wq{m}") for m in range(8)]
            wk_sb = [wpool.tile([128, F], BF16, tag=f"wk{m}", name=f"wk{m}") for m in range(8)]
            wv_sb = [wpool.tile([128, F], BF16, tag=f"wv{m}", name=f"wv{m}") for m in range(8)]
            wo_sb = [wpool.tile([128, DM], BF16, tag=f"wo{f}", name=f"wo{f}") for f in range(4)]
            qT_sb = [big.tile([128, S], BF16, tag=f"qT{f}", name=f"qT{f}") for f in range(4)]
            kT_sb = [big.tile([128, S], BF16, tag=f"kT{f}", name=f"kT{f}") for f in range(4)]
            oT_sb = [big.tile([128, S], BF16, tag=f"oT{f}", name=f"oT{f}") for f in range(4)]
            v_sb = [big.tile([128, NH * (HD + 1)], BF16, tag=f"v{t}", name=f"v{t}") for t in range(KT)]
            bq_sb = wpool.tile([128, 4], F32, tag="bq")
            bk_sb = wpool.tile([128, 4], F32, tag="bk")
            bv_sb = wpool.tile([1, F], F32R, tag="bv")
            ones_sb = wpool.tile([1, 128], F32R, tag="ones")
            bvbc_sb = wpool.tile([128, F], F32, tag="bvbc")

            for m in range(8):
                nc.sync.dma_start(wq_sb[m][:], d_wq[128 * m:128 * (m + 1), :])
                nc.sync.dma_start(wk_sb[m][:], d_wk[128 * m:128 * (m + 1), :])
                nc.sync.dma_start(wv_sb[m][:], d_wv[128 * m:128 * (m + 1), :])
            for f in range(4):
                nc.sync.dma_start(wo_sb[f][:], d_wo[128 * f:128 * (f + 1), :])
            nc.sync.dma_start(bq_sb[:], d_bq.rearrange("(f p) -> p f", p=128))
            nc.sync.dma_start(bk_sb[:], d_bk.rearrange("(f p) -> p f", p=128))
            nc.sync.dma_start(bv_sb[:], d_bv.rearrange("(a f) -> a f", a=1))
            nc.sync.dma_start(ones_sb[:], d_ones[:])

            # bv broadcast over partitions via K=1 matmul (biases are usually
            # zero here, but keep the math general)
            psbv = ps_misc.tile([128, 512], F32, tag="ps", name="psbv")
            nc.tensor.matmul(psbv[:], ones_sb[0:1, :], bv_sb[0:1, :])
            nc.vector.tensor_copy(bvbc_sb[:], psbv[:])

            # ---- projections producing transposed outputs ([feat, tok]).
            # Input chunks are loaded once per token chunk and reused for all
            # four feature tiles; per bank the 8 accumulating matmuls run
            # back-to-back (K-contiguous).
            def proj_load_chunk(src, n):
                chs = []
                for m in range(8):
                    ch = inch.tile([128, 512], BF16, tag="inch")
                    nc.sync.dma_start(
                        ch[:],
                        src[128 * m:128 * (m + 1), 512 * n:512 * (n + 1)],
                    )
                    chs.append(ch)
                return chs

            def proj_piece(w_sb, chs, bias_sb, dst_sb, n, f):
                # one feature tile: 8 K-contiguous accumulating matmuls
                accp = ps_pv.tile([128, 512], F32, tag="po", name="accp")
                for m in range(8):
                    nc.tensor.matmul(
                        accp[:],
                        w_sb[m][:, 128 * f:128 * (f + 1)],
                        chs[m][:],
                        start=(m == 0),
                        stop=(m == 7),
                    )
                with nc.allow_low_precision(reason="bf16 qT/kT store"):
                    nc.vector.tensor_scalar_add(
                        dst_sb[f][:, 512 * n:512 * (n + 1)],
                        accp[:],
                        bias_sb[:, f:f + 1],
                    )

            def k_piece(n, f):
                # loads its own chunk so pieces can be emitted f-major
                chs = proj_load_chunk(d_kt, n)
                proj_piece(wk_sb, chs, bk_sb, kT_sb, n, f)

            # ---- v projection (natural layout, bf16, ones column per head)
            def v_piece(t):
                acc = ps_pv.tile([128, 512], F32, tag="po", name="accv")
                for m in range(8):
                    ch = vtch.tile([128, 128], BF16, tag="vtch")
                    nc.sync.dma_start(
                        ch[:], d_vt[128 * m:128 * (m + 1), 128 * t:128 * (t + 1)]
                    )
                    nc.tensor.matmul(
                        acc[:], ch[:], wv_sb[m][:], start=(m == 0), stop=(m == 7)
                    )
                v3 = v_sb[t][:].rearrange("p (h e) -> p h e", e=HD + 1)
                nc.vector.memset(v3[:, :, HD:HD + 1], 1.0)
                nc.vector.tensor_add(
                    v3[:, :, 0:HD],
                    acc[:].rearrange("p (h e) -> p h e", e=HD),
                    bvbc_sb[:].rearrange("p (h e) -> p h e", e=HD),
                )

            # ---- attention + output projection, q-chunk major; the q
            # projection for chunk n is emitted right before chunk n's
            # attention so it pipelines into the attention stream
            wo_pending = []

            def emit_wo(count):
                for _ in range(count):
                    if not wo_pending:
                        return
                    tt, j = wo_pending.pop(0)
                    tsl = slice(128 * tt, 128 * (tt + 1))
                    pw = ps_misc.tile([128, 512], F32, tag="ps", name="pw")
                    for f in range(4):
                        nc.tensor.matmul(
                            pw[:], oT_sb[f][:, tsl],
                            wo_sb[f][:, 512 * j:512 * (j + 1)],
                            start=(f == 0), stop=(f == 3),
                        )
                    ost = outst.tile([128, 512], F32, tag="outst")
                    nc.vector.tensor_copy(ost[:], pw[:])
                    nc.sync.dma_start(
                        d_part[tsl, 512 * j:512 * (j + 1)], ost[:]
                    )

            def norm_phase1(po):
                # evacuate o' from PSUM right away so the accumulator bank
                # frees for the next group's PV
                oc = ocp.tile([65, 512], F32, tag="oc", name="oc")
                nc.vector.tensor_copy(oc[:], po[0:65, :])
                return oc

            def make_norm(p, qsl, i, oc):
                # broadcast the raw denominator via a K=1 matmul, then the
                # (slow) reciprocal + multiply run from SBUF in DVE slack
                def norm():
                    r0 = 64 * i
                    dn = small.tile([1, 512], F32R, tag="dn", name="dn")
                    nc.vector.tensor_copy(dn[0:1, :], oc[64:65, :])
                    pb = ps_misc.tile([128, 512], F32, tag="ps", name="pb")
                    nc.tensor.matmul(pb[0:64, :], ones_sb[0:1, 0:64], dn[0:1, :])
                    rc = rcp.tile([64, 512], F32, tag="rc", name="rc")
                    with nc.allow_low_precision(reason="f32 reciprocal of bcast"):
                        nc.vector.reciprocal(rc[:], pb[0:64, :])
                    with nc.allow_low_precision(reason="bf16 normalized out"):
                        nc.vector.tensor_mul(
                            oT_sb[p][r0:r0 + 64, qsl], oc[0:64, :], rc[:]
                        )
                return norm

            # minimal serial head: only what attention group (0, 0) needs.
            # Everything else (k/q feature tiles for later pairs, remaining
            # v token tiles, later q chunks, Wo, normalizes) is chased into
            # the attention groups' PE slack.
            pending_norms = []
            pending_norms2 = []
            kchs_cur = proj_load_chunk(d_kt, 0)
            for n4 in range(NQT):
                kchs_next = proj_load_chunk(d_kt, n4 + 1) if n4 + 1 < NQT else None
                for f4 in range(4):
                    proj_piece(wk_sb, kchs_cur, bk_sb, kT_sb, n4, f4)
                kchs_cur = kchs_next
            for t in range(KT):
                v_piece(t)
            qchs = proj_load_chunk(d_qt, 0)
            for f4 in range(4):
                proj_piece(wq_sb, qchs, bq_sb, qT_sb, 0, f4)

            for n in range(NQT):
                if n + 1 < NQT:
                    next_qchs = proj_load_chunk(d_qt, n + 1)
                qsl = slice(512 * n, 512 * (n + 1))
                for p in range(NP):
                    poA = ps_pv.tile([128, 512], F32, tag="po")
                    poB = ps_pv.tile([128, 512], F32, tag="po")

                    def sc_emit(m, p=p, qsl=qsl):
                        scp = ps_sc.tile([128, 1024], F32, tag="sc")
                        ksl = slice(128 * m, 128 * (m + 1))
                        nc.tensor.matmul(
                            scp[:, 0:512], kT_sb[p][0:64, ksl], qT_sb[p][0:64, qsl],
                            tile_position=(0, 0),
                        )
                        nc.tensor.matmul(
                            scp[:, 512:1024], kT_sb[p][64:128, ksl],
                            qT_sb[p][64:128, qsl], tile_position=(64, 0),
                        )
                        ex = exch.tile([128, 1024], BF16, tag="ex")
                        nc.scalar.activation(ex[:], scp[:], EXP, scale=SCALE)
                        return ex

                    exs = {0: sc_emit(0), 1: sc_emit(1)}
                    for m in range(KT):
                        if m + 2 < KT:
                            exs[m + 2] = sc_emit(m + 2)
                        ex = exs.pop(m)
                        nc.tensor.matmul(
                            poA[0:65, :], v_sb[m][:, 130 * p:130 * p + 65],
                            ex[:, 0:512], start=(m == 0), stop=(m == KT - 1),
                        )
                        nc.tensor.matmul(
                            poB[0:65, :], v_sb[m][:, 130 * p + 65:130 * p + 130],
                            ex[:, 512:1024], start=(m == 0), stop=(m == KT - 1),
                        )
                        # previous group's normalizes land just after this
                        # group's pipeline is rolling; Wo units fill PE slack
                        if m in (0, 1) and pending_norms:
                            po_, p_, qsl_, i_ = pending_norms.pop(0)
                            pending_norms2.append(
                                make_norm(p_, qsl_, i_, norm_phase1(po_)))
                        if m in (4, 6) and pending_norms2:
                            pending_norms2.pop(0)()
                        if m in (7, 10, 13):
                            emit_wo(1)
                    if n + 1 < NQT:
                        # next chunk's q projection, one feature tile per group
                        proj_piece(wq_sb, next_qchs, bq_sb, qT_sb, n + 1, p)
                    pending_norms.append((poA, p, qsl, 0))
                    pending_norms.append((poB, p, qsl, 1))
                    if n == NQT - 1:
                        # final chunk: run norms eagerly so the tail is short
                        while pending_norms:
                            po_, p_, qsl_, i_ = pending_norms.pop(0)
                            pending_norms2.append(
                                make_norm(p_, qsl_, i_, norm_phase1(po_)))
                        while pending_norms2:
                            pending_norms2.pop(0)()
                        emit_wo(2)
                # queue this chunk's Wo pieces (interleaved into later groups)
                for t in range(4):
                    for j in range(2):
                        wo_pending.append((4 * n + t, j))
            for entry in pending_norms:
                po_, p_, qsl_, i_ = entry
                pending_norms2.append(make_norm(p_, qsl_, i_, norm_phase1(po_)))
            for nm in pending_norms2:
                nm()
            emit_wo(len(wo_pending))

    _split_multiwaits(nc)
    return nc


_PROGRAM = None


def _get_program():
    global _PROGRAM
    if _PROGRAM is None:
        _PROGRAM = build_program()
    return _PROGRAM


def make_in_maps(Q, K, V, Wq, bq, Wk, bk, Wv, bv, Wo, bo):
    import ml_dtypes
    bf = lambda x: np.asarray(x, dtype=np.float32).astype(ml_dtypes.bfloat16)
    f32 = lambda x: np.asarray(x, dtype=np.float32)
    Q, K, V = bf(Q), bf(K), bf(V)
    Wq, Wk, Wv, Wo = bf(Wq), bf(Wk), bf(Wv), bf(Wo)
    bq, bk, bv = f32(bq), f32(bk), f32(bv)
    ones = np.ones((1, 128), np.float32)
    in_maps = []
    for c in range(N_CORES):
        b, hh = c // 2, c % 2
        fs = slice(F * hh, F * (hh + 1))
        in_maps.append({
            "qt": np.ascontiguousarray(Q[b].T),
            "kt": np.ascontiguousarray(K[b].T),
            "vt": np.ascontiguousarray(V[b].T),
            "wq": np.ascontiguousarray(Wq[:, fs]),
            "wk": np.ascontiguousarray(Wk[:, fs]),
            "wv": np.ascontiguousarray(Wv[:, fs]),
            "wo": np.ascontiguousarray(Wo[fs, :]),
            "bq": np.ascontiguousarray(bq[fs]),
            "bk": np.ascontiguousarray(bk[fs]),
            "bv": np.ascontiguousarray(bv[fs]),
            "ones": ones,
        })
    return in_maps


def kernel(Q, K, V, Wq, bq, Wk, bk, Wv, bv, Wo, bo, _trace=False, _trace_kwargs=None):
    nc = _get_program()
    in_maps = make_in_maps(Q, K, V, Wq, bq, Wk, bk, Wv, bv, Wo, bo)
    res = run_bass_kernel_spmd(
        nc, in_maps, core_ids=list(range(N_CORES)),
        trace=_trace, **(_trace_kwargs or {}),
    )
    parts = [r["part"] for r in res.results]
    out = np.stack([parts[2 * b] + parts[2 * b + 1] for b in range(B)])
    out += np.asarray(bo, dtype=np.float32)[None, None, :]
    if _trace:
        return out, res
    return out



# revision 9
# speedup vs baseline: 1.0848x; 1.0848x over previous
"""Multi-head attention (B=4, S=2048, D=1024, H=16) on 8 trn2 NeuronCores.

Sharding: (batch, head-half) -> one core each. Core c handles batch c//2 and
heads (c%2)*8 .. (c%2)*8+7 (feature columns (c%2)*512 .. +512 of the QKV
projections, rows of Wo). Each core computes its 8 heads' attention and a
partial output projection; the host sums the two partials per batch and adds
the output bias.

v2 schedule (vs the 540us baseline):
  - input DMA is merged into [128, 1024]+ transfers split across the SP and
    Activation HW-DGE queues (the baseline serialized 256 chunk loads on the
    single SP queue at ~600ns each, gating the first 150us),
  - attention starts as soon as pair-0's k/q/v pieces exist; all remaining
    projection work (k f=1..3, v t>=4, q chunks) is chased just-in-time into
    the attention groups' PE slack,
  - the (group, ktile) stream is one flat software pipeline with scores
    issued 2 units ahead across group boundaries so the ACT engine (exp is
    the steady-state bottleneck: 256 x ~1.0us) never drains,
  - softmax denominators of both heads are reciprocal'd in one [2, 512] DVE
    op per group (the baseline spent 107us on [64, 512] reciprocals).
Matmuls run in bf16 (fp32 PSUM accumulation); softmax denominators and
reciprocals stay fp32.
"""
import numpy as np

import concourse.bass as bass
import concourse.tile as tile
from concourse import mybir
from concourse.bass_utils import run_bass_kernel_spmd

F32 = mybir.dt.float32
F32R = mybir.dt.float32r
BF16 = mybir.dt.bfloat16
EXP = mybir.ActivationFunctionType.Exp

B, S, DM, H_TOT = 4, 2048, 1024, 16
F = 512          # features per core (8 heads x 64)
HD = 64          # head dim
NH = 8           # heads per core
NP = 4           # head pairs per core
KT = 16          # k tiles of 128
NQT = 4          # q chunks of 512
SCALE = 0.125    # 1/sqrt(64)
N_CORES = 8

_WAIT_CAP = {"InstEventSemaphore": 2}


def _split_multiwaits(nc):
    """This walrus build accepts 1 sync-wait per instruction (2 on
    EventSemaphore); spread extras over same-engine NOPs placed before."""
    n_spill = 0
    for f in nc.m.functions:
        for bb in f.blocks:
            new = []
            changed = False
            for inst in bb.instructions:
                si = inst.sync_info
                cap = _WAIT_CAP.get(type(inst).__name__, 1)
                if si is not None and len(si.on_wait) > cap:
                    extra = list(si.on_wait[: len(si.on_wait) - cap])
                    del si.on_wait[: len(si.on_wait) - cap]
                    for w in extra:
                        n_spill += 1
                        nop = mybir.InstNoOp(name=f"I-wspill-{n_spill}-{inst.name}")
                        nop.engine = inst.engine
                        nop.sync_info = mybir.SyncInfo(on_wait=[w], on_update=[])
                        new.append(nop)
                    changed = True
                new.append(inst)
            if changed:
                bb.instructions[:] = new
    return n_spill


def build_program():
    nc = bass.Bass("TRN2", target_bir_lowering=False, debug=False, num_devices=1)

    d_qt = nc.dram_tensor("qt", [DM, S], BF16, kind="ExternalInput").ap()
    d_kt = nc.dram_tensor("kt", [DM, S], BF16, kind="ExternalInput").ap()
    d_vt = nc.dram_tensor("vt", [DM, S], BF16, kind="ExternalInput").ap()
    d_wq = nc.dram_tensor("wq", [DM, F], BF16, kind="ExternalInput").ap()
    d_wk = nc.dram_tensor("wk", [DM, F], BF16, kind="ExternalInput").ap()
    d_wv = nc.dram_tensor("wv", [DM, F], BF16, kind="ExternalInput").ap()
    d_wo = nc.dram_tensor("wo", [F, DM], BF16, kind="ExternalInput").ap()
    d_bq = nc.dram_tensor("bq", [F], F32, kind="ExternalInput").ap()
    d_bk = nc.dram_tensor("bk", [F], F32, kind="ExternalInput").ap()
    d_bv = nc.dram_tensor("bv", [F], F32R, kind="ExternalInput").ap()
    d_ones = nc.dram_tensor("ones", [65, 128], F32R, kind="ExternalInput").ap()
    d_part = nc.dram_tensor("part", [S, DM], F32, kind="ExternalOutput").ap()

    with tile.TileContext(nc) as tc:
        with (
            tc.tile_pool(name="wpool", bufs=1) as wpool,
            tc.tile_pool(name="big", bufs=1) as big,
            tc.tile_pool(name="oTp", bufs=2) as oTp,
            tc.tile_pool(name="ktst", bufs=2) as ktst,
            tc.tile_pool(name="qtst", bufs=1) as qtst,
            tc.tile_pool(name="vtst", bufs=2) as vtst,
            tc.tile_pool(name="exch", bufs=3) as exch,
            tc.tile_pool(name="small", bufs=4) as small,
            tc.tile_pool(name="outst", bufs=2) as outst,
            tc.tile_pool(name="rcp", bufs=2) as rcp,
            tc.tile_pool(name="ocp", bufs=4) as ocp,
            tc.tile_pool(name="ps_sc", bufs=2, space="PSUM") as ps_sc,
            tc.tile_pool(name="ps_pv", bufs=3, space="PSUM") as ps_pv,
            tc.tile_pool(name="ps_acc", bufs=1, space="PSUM") as ps_acc,
        ):
            # ---- resident weight tiles (one merged DMA each)
            wq_sb = wpool.tile([128, 8 * F], BF16, tag="wq")
            wk_sb = wpool.tile([128, 8 * F], BF16, tag="wk")
            wv_sb = wpool.tile([128, 8 * F], BF16, tag="wv")
            wo_sb = wpool.tile([128, 4 * DM], BF16, tag="wo")
            bq_sb = wpool.tile([128, 4], F32, tag="bq")
            bk_sb = wpool.tile([128, 4], F32, tag="bk")
            bv_sb = wpool.tile([1, F], F32R, tag="bv")
            ones_sb = wpool.tile([65, 128], F32R, tag="ones")
            bvbc_sb = wpool.tile([128, F], F32, tag="bvbc")
            warm_sb = wpool.tile([1, 2], F32, tag="warm")
            dn2_sb = wpool.tile([65, 512], F32, tag="dn2")

            # big attention tiles: qT/kT hold the full token range per pair;
            # oT rotates per q-chunk (wo consumption finishes within 1 chunk)
            qT_sb = [big.tile([128, S], BF16, tag=f"qT{f}", name=f"qT{f}") for f in range(4)]
            kT_sb = [big.tile([128, S], BF16, tag=f"kT{f}", name=f"kT{f}") for f in range(4)]
            v_sb = [big.tile([128, NH * (HD + 1)], BF16, tag=f"v{t}", name=f"v{t}") for t in range(KT)]

            def blk(msrc, b, h):
                return msrc[128 * b:128 * (b + 1), 1024 * h:1024 * (h + 1)]

            # ---- DMA issue. sync queue: k-path weights + kt + late qt + wo.
            # scalar queue: q/v-path (issued before any ACTIVATE is emitted).
            nc.sync.dma_start(bq_sb[:], d_bq.rearrange("(f p) -> p f", p=128))
            nc.sync.dma_start(bk_sb[:], d_bk.rearrange("(f p) -> p f", p=128))
            nc.sync.dma_start(wk_sb[:].rearrange("p (b c) -> p b c", b=8), d_wk.rearrange("(b r) c -> r b c", r=128))
            kt_t = {}
            for h in range(2):
                for b in range(8):
                    t_ = ktst.tile([128, 1024], BF16, tag=f"kt{b}", name=f"kt{b}h{h}")
                    nc.sync.dma_start(t_[:], blk(d_kt, b, h))
                    kt_t[(b, h)] = t_

            nc.scalar.dma_start(bv_sb[:], d_bv.rearrange("(a f) -> a f", a=1))
            nc.scalar.dma_start(ones_sb[:], d_ones[:])
            nc.scalar.dma_start(wq_sb[:].rearrange("p (b c) -> p b c", b=8), d_wq.rearrange("(b r) c -> r b c", r=128))
            qt_t = {}
            for b in range(8):
                t_ = qtst.tile([128, 1024], BF16, tag=f"qt{b}", name=f"qt{b}h0")
                nc.scalar.dma_start(t_[:], blk(d_qt, b, 0))
                qt_t[(b, 0)] = t_
            nc.scalar.dma_start(wv_sb[:].rearrange("p (b c) -> p b c", b=8), d_wv.rearrange("(b r) c -> r b c", r=128))
            vt_t = {}
            for h in range(2):
                for b in range(8):
                    t_ = vtst.tile([128, 1024], BF16, tag=f"vt{b}", name=f"vt{b}h{h}")
                    nc.scalar.dma_start(t_[:], blk(d_vt, b, h))
                    vt_t[(b, h)] = t_

            # late loads on sync: qt second halves + wo
            for b in range(8):
                t_ = qtst.tile([128, 1024], BF16, tag=f"qt{b}", name=f"qt{b}h1")
                nc.sync.dma_start(t_[:], blk(d_qt, b, 1))
                qt_t[(b, 1)] = t_
            nc.sync.dma_start(wo_sb[:].rearrange("p (b c) -> p b c", b=4), d_wo.rearrange("(b r) c -> r b c", r=128))

            # preload the exp table set while DMA streams
            nc.scalar.activation(warm_sb[:], ones_sb[0:1, 0:2], EXP)

            # denominators of both heads live at partitions 0 and 64 of one
            # [65, 512] tile so a single reciprocal covers both (cost is
            # free-size bound); rows 1..63 are don't-care
            nc.vector.memset(dn2_sb[:], 1.0)

            # bv broadcast over partitions via K=1 matmul
            psbv = ps_acc.tile([128, 512], F32, tag="pacc", name="psbv")
            nc.tensor.matmul(psbv[:], ones_sb[0:1, :], bv_sb[0:1, :])
            nc.vector.tensor_copy(bvbc_sb[:], psbv[:])

            # ---- projection pieces ------------------------------------
            def kq_piece(src_t, w_sb, bias_sb, dst_sb, n, f):
                # one [128 feat, 512 tok] piece: 8 K-contiguous matmuls
                h, o = n // 2, 512 * (n % 2)
                accp = ps_acc.tile([128, 512], F32, tag="pacc", name=f"acc{n}{f}")
                for m in range(8):
                    nc.tensor.matmul(
                        accp[:],
                        w_sb[:, 512 * m + 128 * f:512 * m + 128 * (f + 1)],
                        src_t[(m, h)][:, o:o + 512],
                        start=(m == 0), stop=(m == 7),
                    )
                with nc.allow_low_precision(reason="bf16 qT/kT store"):
                    nc.vector.tensor_scalar_add(
                        dst_sb[f][:, 512 * n:512 * (n + 1)],
                        accp[:],
                        bias_sb[:, f:f + 1],
                    )

            def v_piece(t):
                h, o = t // 8, 128 * (t % 8)
                acc = ps_acc.tile([128, 512], F32, tag="pacc", name=f"accv{t}")
                for m in range(8):
                    nc.tensor.matmul(
                        acc[:], vt_t[(m, h)][:, o:o + 128],
                        wv_sb[:, 512 * m:512 * (m + 1)],
                        start=(m == 0), stop=(m == 7),
                    )
                v3 = v_sb[t][:].rearrange("p (h e) -> p h e", e=HD + 1)
                nc.vector.memset(v3[:, :, HD:HD + 1], 1.0)
                nc.vector.tensor_add(
                    v3[:, :, 0:HD],
                    acc[:].rearrange("p (h e) -> p h e", e=HD),
                    bvbc_sb[:].rearrange("p (h e) -> p h e", e=HD),
                )

            # ---- wo output projection pieces (chased) ------------------
            wo_pending = []
            oT_cur = [None] * 4  # current chunk's [128, 512] oT tiles

            def emit_wo(count):
                for _ in range(count):
                    if not wo_pending:
                        return
                    oTs, tt, j = wo_pending.pop(0)
                    pw = ps_acc.tile([128, 512], F32, tag="pacc", name="pw")
                    for f in range(4):
                        nc.tensor.matmul(
                            pw[:], oTs[f][:, 128 * (tt % 4):128 * (tt % 4 + 1)],
                            wo_sb[:, 1024 * f + 512 * j:1024 * f + 512 * (j + 1)],
                            start=(f == 0), stop=(f == 3),
                        )
                    ost = outst.tile([128, 512], F32, tag="outst")
                    nc.vector.tensor_copy(ost[:], pw[:])
                    nc.sync.dma_start(
                        d_part[128 * tt:128 * (tt + 1), 512 * j:512 * (j + 1)], ost[:]
                    )

            # ---- per-group normalization, pipelined in 4 phases --------
            # state: (poA, poB, oTs, p) of the previous group
            def norm_p1(po, nm):
                oc = ocp.tile([65, 512], F32, tag="oc", name=nm)
                nc.vector.tensor_copy(oc[:], po[0:65, :])
                return oc

            def norm_recip(ocA, ocB):
                nc.vector.tensor_copy(dn2_sb[0:1, :], ocA[64:65, :])
                nc.vector.tensor_copy(dn2_sb[64:65, :], ocB[64:65, :])
                rc2 = rcp.tile([65, 512], F32R, tag="rc", name="rc2")
                with nc.allow_low_precision(reason="f32 reciprocal of denom"):
                    nc.vector.reciprocal(rc2[:], dn2_sb[:])
                return rc2

            def norm_bcast(rc2, i):
                # broadcast head i's reciprocal row to a base-0 [64, 512]
                pb = ps_acc.tile([128, 512], F32, tag="pacc", name=f"pb{i}")
                r = 64 * i
                nc.tensor.matmul(pb[0:64, :], ones_sb[r:r + 1, 0:64], rc2[r:r + 1, :])
                return pb

            def norm_mul(oc, pb, oTf, i):
                with nc.allow_low_precision(reason="bf16 normalized out"):
                    nc.vector.tensor_mul(
                        oTf[64 * i:64 * i + 64, :], oc[0:64, :], pb[0:64, :],
                    )

            # ---- minimal serial head: just what group (0,0) m=0..3 needs
            for n4 in range(NQT):
                kq_piece(kt_t, wk_sb, bk_sb, kT_sb, n4, 0)
            kq_piece(qt_t, wq_sb, bq_sb, qT_sb, 0, 0)
            for t in range(4):
                v_piece(t)

            # ---- flat attention pipeline over all (group, m) units -----
            groups = [(n, p) for n in range(NQT) for p in range(NP)]

            def sc_emit(gi, m):
                n, p = groups[gi]
                qsl = slice(512 * n, 512 * (n + 1))
                scp = ps_sc.tile([128, 1024], F32, tag="sc")
                ksl = slice(128 * m, 128 * (m + 1))
                nc.tensor.matmul(
                    scp[:, 0:512], kT_sb[p][0:64, ksl], qT_sb[p][0:64, qsl],
                    tile_position=(0, 0),
                )
                nc.tensor.matmul(
                    scp[:, 512:1024], kT_sb[p][64:128, ksl],
                    qT_sb[p][64:128, qsl], tile_position=(64, 0),
                )
                ex = exch.tile([128, 1024], BF16, tag="ex")
                nc.scalar.activation(ex[:], scp[:], EXP, scale=SCALE)
                return ex

            NU = len(groups) * KT
            exq = {}
            exq[0] = sc_emit(0, 0)
            exq[1] = sc_emit(0, 1)
            prev_state = None   # (ocA?, ... ) pipeline of previous group
            norm_st = {}

            for gi, (n, p) in enumerate(groups):
                # fresh oT tiles at the start of each chunk
                if p == 0:
                    oT_cur = [oTp.tile([128, 512], BF16, tag=f"oT{f}", name=f"oT{f}c{n}")
                              for f in range(4)]
                oTs = oT_cur
                poA = ps_pv.tile([128, 512], F32, tag="po", name=f"poA{gi}")
                poB = ps_pv.tile([128, 512], F32, tag="po", name=f"poB{gi}")
                for m in range(KT):
                    u = gi * KT + m
                    if u + 2 < NU:
                        exq[u + 2] = sc_emit((u + 2) // KT, (u + 2) % KT)
                    ex = exq.pop(u)
                    nc.tensor.matmul(
                        poA[0:65, :], v_sb[m][:, 130 * p:130 * p + 65],
                        ex[:, 0:512], start=(m == 0), stop=(m == KT - 1),
                    )
                    nc.tensor.matmul(
                        poB[0:65, :], v_sb[m][:, 130 * p + 65:130 * p + 130],
                        ex[:, 512:1024], start=(m == 0), stop=(m == KT - 1),
                    )
                    # ---- previous group's norm pipeline in this group's slack
                    if prev_state is not None:
                        pA, pB, pT, pp = prev_state
                        if m == 0:
                            norm_st["ocA"] = norm_p1(pA, f"ocA{gi}")
                        elif m == 1:
                            norm_st["ocB"] = norm_p1(pB, f"ocB{gi}")
                        elif m == 2:
                            norm_st["rc2"] = norm_recip(norm_st["ocA"], norm_st["ocB"])
                        elif m == 5:
                            norm_st["pbA"] = norm_bcast(norm_st["rc2"], 0)
                        elif m == 6:
                            norm_mul(norm_st["ocA"], norm_st["pbA"], pT[pp], 0)
                        elif m == 7:
                            norm_st["pbB"] = norm_bcast(norm_st["rc2"], 1)
                        elif m == 8:
                            norm_mul(norm_st["ocB"], norm_st["pbB"], pT[pp], 1)
                    # ---- chased projection pieces (just-in-time, 1-group lead)
                    if n == 0:
                        if p == 0 and 3 <= m <= 14:
                            v_piece(m + 1)  # v tile m+1 one slot before its PV
                        if p == 0 and m == 10:
                            kq_piece(kt_t, wk_sb, bk_sb, kT_sb, 0, 1)
                        if p >= 1 and m in (0, 4, 8):
                            kq_piece(kt_t, wk_sb, bk_sb, kT_sb, 1 + m // 4, p)
                        if p < 3 and m == 12:
                            kq_piece(qt_t, wq_sb, bq_sb, qT_sb, 0, p + 1)
                        if p < 2 and m == 14:
                            kq_piece(kt_t, wk_sb, bk_sb, kT_sb, 0, p + 2)
                    # ---- wo pieces
                    if m in (9, 11, 13):
                        emit_wo(1)
                # next chunk's q piece for this pair
                if n + 1 < NQT:
                    kq_piece(qt_t, wq_sb, bq_sb, qT_sb, n + 1, p)
                prev_state = (poA, poB, oTs, p)
                # queue this chunk's wo pieces once the chunk's groups are done
                if p == NP - 1:
                    for t in range(4):
                        for j in range(2):
                            wo_pending.append((oTs, 4 * n + t, j))

            # ---- tail: last group's norms + remaining wo
            pA, pB, pT, pp = prev_state
            ocA = norm_p1(pA, "ocAz")
            ocB = norm_p1(pB, "ocBz")
            rc2 = norm_recip(ocA, ocB)
            pbA = norm_bcast(rc2, 0)
            norm_mul(ocA, pbA, pT[pp], 0)
            pbB = norm_bcast(rc2, 1)
            norm_mul(ocB, pbB, pT[pp], 1)
            emit_wo(len(wo_pending))

    _split_multiwaits(nc)
    return nc


_PROGRAM = None


def _get_program():
    global _PROGRAM
    if _PROGRAM is None:
        _PROGRAM = build_program()
    return _PROGRAM


def make_in_maps(Q, K, V, Wq, bq, Wk, bk, Wv, bv, Wo, bo):
    import ml_dtypes
    bf = lambda x: np.asarray(x, dtype=np.float32).astype(ml_dtypes.bfloat16)
    f32 = lambda x: np.asarray(x, dtype=np.float32)
    Q, K, V = bf(Q), bf(K), bf(V)
    Wq, Wk, Wv, Wo = bf(Wq), bf(Wk), bf(Wv), bf(Wo)
    bq, bk, bv = f32(bq), f32(bk), f32(bv)
    ones = np.ones((65, 128), np.float32)
    in_maps = []
    for c in range(N_CORES):
        b, hh = c // 2, c % 2
        fs = slice(F * hh, F * (hh + 1))
        in_maps.append({
            "qt": np.ascontiguousarray(Q[b].T),
            "kt": np.ascontiguousarray(K[b].T),
            "vt": np.ascontiguousarray(V[b].T),
            "wq": np.ascontiguousarray(Wq[:, fs]),
            "wk": np.ascontiguousarray(Wk[:, fs]),
            "wv": np.ascontiguousarray(Wv[:, fs]),
            "wo": np.ascontiguousarray(Wo[fs, :]),
            "bq": np.ascontiguousarray(bq[fs]),
            "bk": np.ascontiguousarray(bk[fs]),
            "bv": np.ascontiguousarray(bv[fs]),
            "ones": ones,
        })
    return in_maps


def kernel(Q, K, V, Wq, bq, Wk, bk, Wv, bv, Wo, bo, _trace=False, _trace_kwargs=None):
    nc = _get_program()
    in_maps = make_in_maps(Q, K, V, Wq, bq, Wk, bk, Wv, bv, Wo, bo)
    res = run_bass_kernel_spmd(
        nc, in_maps, core_ids=list(range(N_CORES)),
        trace=_trace, **(_trace_kwargs or {}),
    )
    parts = [r["part"] for r in res.results]
    out = np.stack([parts[2 * b] + parts[2 * b + 1] for b in range(B)])
    out += np.asarray(bo, dtype=np.float32)[None, None, :]
    if _trace:
        return out, res
    return out


# revision 11
# speedup vs baseline: 1.1501x; 1.0602x over previous
"""Multi-head attention (B=4, S=2048, D=1024, H=16) on 8 trn2 NeuronCores.

Sharding: (batch, head-half) -> one core each. Core c handles batch c//2 and
heads (c%2)*8 .. (c%2)*8+7 (feature columns (c%2)*512 .. +512 of the QKV
projections, rows of Wo). Each core computes its 8 heads' attention and a
partial output projection; the host sums the two partials per batch and adds
the output bias.

v2 schedule (vs the 540us baseline):
  - input DMA is merged into [128, 1024]+ transfers split across the SP and
    Activation HW-DGE queues (the baseline serialized 256 chunk loads on the
    single SP queue at ~600ns each, gating the first 150us),
  - attention starts as soon as pair-0's k/q/v pieces exist; all remaining
    projection work (k f=1..3, v t>=4, q chunks) is chased just-in-time into
    the attention groups' PE slack,
  - the (group, ktile) stream is one flat software pipeline with scores
    issued 2 units ahead across group boundaries so the ACT engine (exp is
    the steady-state bottleneck: 256 x ~1.0us) never drains,
  - softmax denominators of both heads are reciprocal'd in one [2, 512] DVE
    op per group (the baseline spent 107us on [64, 512] reciprocals).
Matmuls run in bf16 (fp32 PSUM accumulation); softmax denominators and
reciprocals stay fp32.
"""
import numpy as np

import concourse.bass as bass
import concourse.tile as tile
from concourse import mybir
from concourse.bass_utils import run_bass_kernel_spmd

F32 = mybir.dt.float32
F32R = mybir.dt.float32r
BF16 = mybir.dt.bfloat16
EXP = mybir.ActivationFunctionType.Exp

B, S, DM, H_TOT = 4, 2048, 1024, 16
F = 512          # features per core (8 heads x 64)
HD = 64          # head dim
NH = 8           # heads per core
NP = 4           # head pairs per core
KT = 16          # k tiles of 128
NQT = 4          # q chunks of 512
SCALE = 0.125    # 1/sqrt(64)
N_CORES = 8

_WAIT_CAP = {"InstEventSemaphore": 2}


def _split_multiwaits(nc):
    """This walrus build accepts 1 sync-wait per instruction (2 on
    EventSemaphore); spread extras over same-engine NOPs placed before."""
    n_spill = 0
    for f in nc.m.functions:
        for bb in f.blocks:
            new = []
            changed = False
            for inst in bb.instructions:
                si = inst.sync_info
                cap = _WAIT_CAP.get(type(inst).__name__, 1)
                if si is not None and len(si.on_wait) > cap:
                    extra = list(si.on_wait[: len(si.on_wait) - cap])
                    del si.on_wait[: len(si.on_wait) - cap]
                    for w in extra:
                        n_spill += 1
                        nop = mybir.InstNoOp(name=f"I-wspill-{n_spill}-{inst.name}")
                        nop.engine = inst.engine
                        nop.sync_info = mybir.SyncInfo(on_wait=[w], on_update=[])
                        new.append(nop)
                    changed = True
                new.append(inst)
            if changed:
                bb.instructions[:] = new
    return n_spill


def build_program():
    nc = bass.Bass("TRN2", target_bir_lowering=False, debug=False, num_devices=1)

    d_qt = nc.dram_tensor("qt", [DM, S], BF16, kind="ExternalInput").ap()
    d_kt = nc.dram_tensor("kt", [DM, S], BF16, kind="ExternalInput").ap()
    d_vt = nc.dram_tensor("vt", [DM, S], BF16, kind="ExternalInput").ap()
    d_wq = nc.dram_tensor("wq", [DM, F], BF16, kind="ExternalInput").ap()
    d_wk = nc.dram_tensor("wk", [DM, F], BF16, kind="ExternalInput").ap()
    d_wv = nc.dram_tensor("wv", [DM, F], BF16, kind="ExternalInput").ap()
    d_wo = nc.dram_tensor("wo", [F, DM], BF16, kind="ExternalInput").ap()
    d_bq = nc.dram_tensor("bq", [F], F32, kind="ExternalInput").ap()
    d_bk = nc.dram_tensor("bk", [F], F32, kind="ExternalInput").ap()
    d_bv = nc.dram_tensor("bv", [F], F32R, kind="ExternalInput").ap()
    d_ones = nc.dram_tensor("ones", [65, 128], F32R, kind="ExternalInput").ap()
    d_part = nc.dram_tensor("part", [S, DM], F32, kind="ExternalOutput").ap()

    with tile.TileContext(nc) as tc:
        with (
            tc.tile_pool(name="wpool", bufs=1) as wpool,
            tc.tile_pool(name="big", bufs=1) as big,
            tc.tile_pool(name="oTp", bufs=2) as oTp,
            tc.tile_pool(name="ktst", bufs=2) as ktst,
            tc.tile_pool(name="qtst", bufs=1) as qtst,
            tc.tile_pool(name="vtst", bufs=2) as vtst,
            tc.tile_pool(name="exch", bufs=3) as exch,
            tc.tile_pool(name="small", bufs=4) as small,
            tc.tile_pool(name="outst", bufs=2) as outst,
            tc.tile_pool(name="rcp", bufs=2) as rcp,
            tc.tile_pool(name="ocp", bufs=4) as ocp,
            tc.tile_pool(name="ps_sc", bufs=2, space="PSUM") as ps_sc,
            tc.tile_pool(name="ps_pv", bufs=2, space="PSUM") as ps_pv,
            tc.tile_pool(name="ps_acc", bufs=2, space="PSUM") as ps_acc,
        ):
            # ---- resident weight tiles (one merged DMA each)
            wq_sb = wpool.tile([128, 8 * F], BF16, tag="wq")
            wk_sb = wpool.tile([128, 8 * F], BF16, tag="wk")
            wv_sb = wpool.tile([128, 8 * F], BF16, tag="wv")
            wo_sb = wpool.tile([128, 4 * DM], BF16, tag="wo")
            bq_sb = wpool.tile([128, 4], F32, tag="bq")
            bk_sb = wpool.tile([128, 4], F32, tag="bk")
            bv_sb = wpool.tile([1, F], F32R, tag="bv")
            ones_sb = wpool.tile([65, 128], F32R, tag="ones")
            bvbc_sb = wpool.tile([128, F], F32, tag="bvbc")
            warm_sb = wpool.tile([1, 2], F32, tag="warm")
            dn2_sb = wpool.tile([65, 512], F32, tag="dn2")

            # big attention tiles: qT/kT hold the full token range per pair;
            # oT rotates per q-chunk (wo consumption finishes within 1 chunk)
            qT_sb = [big.tile([128, S], BF16, tag=f"qT{f}", name=f"qT{f}") for f in range(4)]
            kT_sb = [big.tile([128, S], BF16, tag=f"kT{f}", name=f"kT{f}") for f in range(4)]
            v_sb = [big.tile([128, NH * (HD + 1)], BF16, tag=f"v{t}", name=f"v{t}") for t in range(KT)]

            def blk(msrc, b, h):
                return msrc[128 * b:128 * (b + 1), 1024 * h:1024 * (h + 1)]

            # ---- DMA issue on two parallel queues: sync (HW DGE) carries
            # the scores-critical path (wk/kt/qt-h0), gpsimd (SW DGE) the
            # PV path (wq/wv/vt).  Per-queue transfers serialize, and the
            # scalar engine must carry NO dma instructions (they would FIFO
            # ahead of every ACTIVATE).
            nc.sync.dma_start(ones_sb[:], d_ones[:])
            nc.sync.dma_start(bq_sb[:], d_bq.rearrange("(f p) -> p f", p=128))
            nc.sync.dma_start(bk_sb[:], d_bk.rearrange("(f p) -> p f", p=128))
            nc.sync.dma_start(wk_sb[:].rearrange("p (b c) -> p b c", b=8), d_wk.rearrange("(b r) c -> r b c", r=128))
            nc.gpsimd.dma_start(bv_sb[:], d_bv.rearrange("(a f) -> a f", a=1))
            nc.gpsimd.dma_start(wq_sb[:].rearrange("p (b c) -> p b c", b=8), d_wq.rearrange("(b r) c -> r b c", r=128))
            nc.gpsimd.dma_start(wv_sb[:].rearrange("p (b c) -> p b c", b=8), d_wv.rearrange("(b r) c -> r b c", r=128))
            kt_t = {}
            qt_t = {}
            vt_t = {}
            for b in range(8):
                t_ = ktst.tile([128, 1024], BF16, tag=f"kt{b}", name=f"kt{b}h0")
                nc.sync.dma_start(t_[:], blk(d_kt, b, 0))
                kt_t[(b, 0)] = t_
                t_ = qtst.tile([128, 1024], BF16, tag=f"qt{b}", name=f"qt{b}h0")
                nc.sync.dma_start(t_[:], blk(d_qt, b, 0))
                qt_t[(b, 0)] = t_
                t_ = vtst.tile([128, 1024], BF16, tag=f"vt{b}", name=f"vt{b}h0")
                nc.gpsimd.dma_start(t_[:], blk(d_vt, b, 0))
                vt_t[(b, 0)] = t_
            for b in range(8):
                t_ = ktst.tile([128, 1024], BF16, tag=f"kt{b}", name=f"kt{b}h1")
                nc.sync.dma_start(t_[:], blk(d_kt, b, 1))
                kt_t[(b, 1)] = t_
                t_ = vtst.tile([128, 1024], BF16, tag=f"vt{b}", name=f"vt{b}h1")
                nc.gpsimd.dma_start(t_[:], blk(d_vt, b, 1))
                vt_t[(b, 1)] = t_
            nc.sync.dma_start(wo_sb[:].rearrange("p (b c) -> p b c", b=4), d_wo.rearrange("(b r) c -> r b c", r=128))
            for b in range(8):
                t_ = qtst.tile([128, 1024], BF16, tag=f"qt{b}", name=f"qt{b}h1")
                nc.gpsimd.dma_start(t_[:], blk(d_qt, b, 1))
                qt_t[(b, 1)] = t_

            # preload the exp table set while DMA streams
            nc.scalar.activation(warm_sb[:], ones_sb[0:1, 0:2], EXP)

            # denominators of both heads live at partitions 0 and 64 of one
            # [65, 512] tile so a single reciprocal covers both (cost is
            # free-size bound); rows 1..63 are don't-care
            nc.vector.memset(dn2_sb[:], 1.0)

            # bv broadcast over partitions via K=1 matmul
            psbv = ps_acc.tile([128, 512], F32, tag="pacc", name="psbv")
            nc.tensor.matmul(psbv[:], ones_sb[0:1, :], bv_sb[0:1, :])
            nc.vector.tensor_copy(bvbc_sb[:], psbv[:])

            # ---- projection pieces ------------------------------------
            def kq_piece(src_t, w_sb, bias_sb, dst_sb, n, f):
                # one [128 feat, 512 tok] piece: 8 K-contiguous matmuls
                h, o = n // 2, 512 * (n % 2)
                accp = ps_acc.tile([128, 512], F32, tag="pacc", name=f"acc{n}{f}")
                for m in range(8):
                    nc.tensor.matmul(
                        accp[:],
                        w_sb[:, 512 * m + 128 * f:512 * m + 128 * (f + 1)],
                        src_t[(m, h)][:, o:o + 512],
                        start=(m == 0), stop=(m == 7),
                    )
                with nc.allow_low_precision(reason="bf16 qT/kT store"):
                    nc.vector.tensor_scalar_add(
                        dst_sb[f][:, 512 * n:512 * (n + 1)],
                        accp[:],
                        bias_sb[:, f:f + 1],
                    )

            def v_piece(t):
                h, o = t // 8, 128 * (t % 8)
                acc = ps_acc.tile([128, 512], F32, tag="pacc", name=f"accv{t}")
                for m in range(8):
                    nc.tensor.matmul(
                        acc[:], vt_t[(m, h)][:, o:o + 128],
                        wv_sb[:, 512 * m:512 * (m + 1)],
                        start=(m == 0), stop=(m == 7),
                    )
                v3 = v_sb[t][:].rearrange("p (h e) -> p h e", e=HD + 1)
                nc.vector.memset(v3[:, :, HD:HD + 1], 1.0)
                nc.vector.tensor_add(
                    v3[:, :, 0:HD],
                    acc[:].rearrange("p (h e) -> p h e", e=HD),
                    bvbc_sb[:].rearrange("p (h e) -> p h e", e=HD),
                )

            # ---- wo output projection pieces (chased) ------------------
            wo_pending = []
            oT_cur = [None] * 4  # current chunk's [128, 512] oT tiles
            wo_ost = {}
            wo_nq = [0]

            def emit_wo(count):
                # one call = one [128, 512] half; the merged [128, 1024] store
                # fires after the second half, alternating sync/gpsimd queues
                for _ in range(count):
                    if not wo_pending:
                        return
                    oTs, tt, j = wo_pending.pop(0)
                    pw = ps_acc.tile([128, 512], F32, tag="pacc", name="pw")
                    for f in range(4):
                        nc.tensor.matmul(
                            pw[:], oTs[f][:, 128 * (tt % 4):128 * (tt % 4 + 1)],
                            wo_sb[:, 1024 * f + 512 * j:1024 * f + 512 * (j + 1)],
                            start=(f == 0), stop=(f == 3),
                        )
                    if tt not in wo_ost:
                        wo_ost[tt] = outst.tile([128, 1024], F32, tag="outst", name=f"ost{tt}")
                    ost = wo_ost[tt]
                    nc.vector.tensor_copy(ost[:, 512 * j:512 * (j + 1)], pw[:])
                    if j == 1:
                        eng = nc.sync if wo_nq[0] % 2 == 0 else nc.gpsimd
                        wo_nq[0] += 1
                        eng.dma_start(
                            d_part[128 * tt:128 * (tt + 1), :], wo_ost.pop(tt)[:]
                        )

            # ---- per-group normalization, pipelined in 4 phases --------
            # state: (poA, poB, oTs, p) of the previous group
            def norm_p1(po, nm):
                oc = ocp.tile([65, 512], F32, tag="oc", name=nm)
                nc.vector.tensor_copy(oc[:], po[0:65, :])
                return oc

            def norm_recip(ocA, ocB):
                nc.vector.tensor_copy(dn2_sb[0:1, :], ocA[64:65, :])
                nc.vector.tensor_copy(dn2_sb[64:65, :], ocB[64:65, :])
                rc2 = rcp.tile([65, 512], F32R, tag="rc", name="rc2")
                with nc.allow_low_precision(reason="f32 reciprocal of denom"):
                    nc.vector.reciprocal(rc2[:], dn2_sb[:])
                return rc2

            def norm_bcast(rc2, i):
                # broadcast head i's reciprocal row to a base-0 [64, 512]
                pb = ps_acc.tile([128, 512], F32, tag="pacc", name=f"pb{i}")
                r = 64 * i
                nc.tensor.matmul(pb[0:64, :], ones_sb[r:r + 1, 0:64], rc2[r:r + 1, :])
                return pb

            def norm_mul(oc, pb, oTf, i):
                with nc.allow_low_precision(reason="bf16 normalized out"):
                    nc.vector.tensor_mul(
                        oTf[64 * i:64 * i + 64, :], oc[0:64, :], pb[0:64, :],
                    )

            # ---- minimal serial head: just what group (0,0) m=0..3 needs
            for n4 in range(NQT):
                kq_piece(kt_t, wk_sb, bk_sb, kT_sb, n4, 0)
            kq_piece(qt_t, wq_sb, bq_sb, qT_sb, 0, 0)
            for t in range(4):
                v_piece(t)

            # ---- flat attention pipeline over all (group, m) units -----
            groups = [(n, p) for n in range(NQT) for p in range(NP)]

            def sc_emit(gi, m):
                n, p = groups[gi]
                qsl = slice(512 * n, 512 * (n + 1))
                scp = ps_sc.tile([128, 1024], F32, tag="sc")
                ksl = slice(128 * m, 128 * (m + 1))
                nc.tensor.matmul(
                    scp[:, 0:512], kT_sb[p][0:64, ksl], qT_sb[p][0:64, qsl],
                    tile_position=(0, 0),
                )
                nc.tensor.matmul(
                    scp[:, 512:1024], kT_sb[p][64:128, ksl],
                    qT_sb[p][64:128, qsl], tile_position=(64, 0),
                )
                ex = exch.tile([128, 1024], BF16, tag="ex")
                nc.scalar.activation(ex[:], scp[:], EXP, scale=SCALE)
                return ex

            NU = len(groups) * KT
            exq = {}
            exq[0] = sc_emit(0, 0)
            exq[1] = sc_emit(0, 1)
            prev_state = None   # (ocA?, ... ) pipeline of previous group
            norm_st = {}

            for gi, (n, p) in enumerate(groups):
                # fresh oT tiles at the start of each chunk
                if p == 0:
                    oT_cur = [oTp.tile([128, 512], BF16, tag=f"oT{f}", name=f"oT{f}c{n}")
                              for f in range(4)]
                oTs = oT_cur
                poA = ps_pv.tile([128, 512], F32, tag="po", name=f"poA{gi}")
                poB = ps_pv.tile([128, 512], F32, tag="po", name=f"poB{gi}")
                for m in range(KT):
                    u = gi * KT + m
                    if u + 2 < NU:
                        exq[u + 2] = sc_emit((u + 2) // KT, (u + 2) % KT)
                    ex = exq.pop(u)
                    nc.tensor.matmul(
                        poA[0:65, :], v_sb[m][:, 130 * p:130 * p + 65],
                        ex[:, 0:512], start=(m == 0), stop=(m == KT - 1),
                    )
                    nc.tensor.matmul(
                        poB[0:65, :], v_sb[m][:, 130 * p + 65:130 * p + 130],
                        ex[:, 512:1024], start=(m == 0), stop=(m == KT - 1),
                    )
                    # ---- previous group's norm pipeline in this group's slack
                    if prev_state is not None:
                        pA, pB, pT, pp = prev_state
                        if m == 0:
                            norm_st["ocA"] = norm_p1(pA, f"ocA{gi}")
                        elif m == 1:
                            norm_st["ocB"] = norm_p1(pB, f"ocB{gi}")
                        elif m == 2:
                            norm_st["rc2"] = norm_recip(norm_st["ocA"], norm_st["ocB"])
                        elif m == 5:
                            norm_st["pbA"] = norm_bcast(norm_st["rc2"], 0)
                        elif m == 6:
                            norm_mul(norm_st["ocA"], norm_st["pbA"], pT[pp], 0)
                        elif m == 7:
                            norm_st["pbB"] = norm_bcast(norm_st["rc2"], 1)
                        elif m == 8:
                            norm_mul(norm_st["ocB"], norm_st["pbB"], pT[pp], 1)
                    # ---- chased projection pieces (just-in-time, 1-group lead)
                    if n == 0:
                        if p == 0 and 3 <= m <= 14:
                            v_piece(m + 1)  # v tile m+1 one slot before its PV
                        if p == 0 and m == 10:
                            kq_piece(kt_t, wk_sb, bk_sb, kT_sb, 0, 1)
                        if p >= 1 and m in (0, 4, 8):
                            kq_piece(kt_t, wk_sb, bk_sb, kT_sb, 1 + m // 4, p)
                        if p < 3 and m == 12:
                            kq_piece(qt_t, wq_sb, bq_sb, qT_sb, 0, p + 1)
                        if p < 2 and m == 14:
                            kq_piece(kt_t, wk_sb, bk_sb, kT_sb, 0, p + 2)
                    # ---- wo pieces
                    if m in (9, 11, 13):
                        emit_wo(1)
                # next chunk's q piece for this pair
                if n + 1 < NQT:
                    kq_piece(qt_t, wq_sb, bq_sb, qT_sb, n + 1, p)
                prev_state = (poA, poB, oTs, p)
                # queue this chunk's wo pieces once the chunk's groups are done
                if p == NP - 1:
                    for t in range(4):
                        for j in range(2):
                            wo_pending.append((oTs, 4 * n + t, j))
                            

            # ---- tail: last group's norms + remaining wo
            pA, pB, pT, pp = prev_state
            ocA = norm_p1(pA, "ocAz")
            ocB = norm_p1(pB, "ocBz")
            rc2 = norm_recip(ocA, ocB)
            pbA = norm_bcast(rc2, 0)
            norm_mul(ocA, pbA, pT[pp], 0)
            pbB = norm_bcast(rc2, 1)
            norm_mul(ocB, pbB, pT[pp], 1)
            emit_wo(len(wo_pending))

    _split_multiwaits(nc)
    return nc


_PROGRAM = None


def _get_program():
    global _PROGRAM
    if _PROGRAM is None:
        _PROGRAM = build_program()
    return _PROGRAM


def make_in_maps(Q, K, V, Wq, bq, Wk, bk, Wv, bv, Wo, bo):
    import ml_dtypes
    bf = lambda x: np.asarray(x, dtype=np.float32).astype(ml_dtypes.bfloat16)
    f32 = lambda x: np.asarray(x, dtype=np.float32)
    Q, K, V = bf(Q), bf(K), bf(V)
    Wq, Wk, Wv, Wo = bf(Wq), bf(Wk), bf(Wv), bf(Wo)
    bq, bk, bv = f32(bq), f32(bk), f32(bv)
    ones = np.ones((65, 128), np.float32)
    in_maps = []
    for c in range(N_CORES):
        b, hh = c // 2, c % 2
        fs = slice(F * hh, F * (hh + 1))
        in_maps.append({
            "qt": np.ascontiguousarray(Q[b].T),
            "kt": np.ascontiguousarray(K[b].T),
            "vt": np.ascontiguousarray(V[b].T),
            "wq": np.ascontiguousarray(Wq[:, fs]),
            "wk": np.ascontiguousarray(Wk[:, fs]),
            "wv": np.ascontiguousarray(Wv[:, fs]),
            "wo": np.ascontiguousarray(Wo[fs, :]),
            "bq": np.ascontiguousarray(bq[fs]),
            "bk": np.ascontiguousarray(bk[fs]),
            "bv": np.ascontiguousarray(bv[fs]),
            "ones": ones,
        })
    return in_maps


def kernel(Q, K, V, Wq, bq, Wk, bk, Wv, bv, Wo, bo, _trace=False, _trace_kwargs=None):
    nc = _get_program()
    in_maps = make_in_maps(Q, K, V, Wq, bq, Wk, bk, Wv, bv, Wo, bo)
    res = run_bass_kernel_spmd(
        nc, in_maps, core_ids=list(range(N_CORES)),
        trace=_trace, **(_trace_kwargs or {}),
    )
    parts = [r["part"] for r in res.results]
    out = np.stack([parts[2 * b] + parts[2 * b + 1] for b in range(B)])
    out += np.asarray(bo, dtype=np.float32)[None, None, :]
    if _trace:
        return out, res
    return out


# revision 12
# speedup vs baseline: 1.1879x; 1.0329x over previous
"""Multi-head attention (B=4, S=2048, D=1024, H=16) on 8 trn2 NeuronCores.

Sharding: (batch, head-half) -> one core each. Core c handles batch c//2 and
heads (c%2)*8 .. (c%2)*8+7 (feature columns (c%2)*512 .. +512 of the QKV
projections, rows of Wo). Each core computes its 8 heads' attention and a
partial output projection; the host sums the two partials per batch and adds
the output bias.

v4 schedule (vs the 540us baseline):
  - steady state is paced by the ACT engine (256 exps x ~1.05us); scores are
    issued 3 units ahead across group boundaries so ACT rides through PE
    bursts, and all chased projection work is split into 4-matmul halves
    spread over adjacent slots,
  - input DMA: quarter-granularity [128, 512] transfers in exact consumption
    order, split over the two usable DGE queues (sync HW + gpsimd SW; the
    scalar engine carries none, its FIFO would block ACTIVATEs),
  - softmax denominators of both heads are packed at partitions 0/64 of one
    [65, 512] tile -> a single DVE reciprocal per group,
  - wo partials merge into [128, 1024] stores alternating between queues.
Matmuls run in bf16 (fp32 PSUM accumulation); softmax denominators and
reciprocals stay fp32.
"""
import numpy as np

import concourse.bass as bass
import concourse.tile as tile
from concourse import mybir
from concourse.bass_utils import run_bass_kernel_spmd

F32 = mybir.dt.float32
F32R = mybir.dt.float32r
BF16 = mybir.dt.bfloat16
EXP = mybir.ActivationFunctionType.Exp

B, S, DM, H_TOT = 4, 2048, 1024, 16
F = 512          # features per core (8 heads x 64)
HD = 64          # head dim
NH = 8           # heads per core
NP = 4           # head pairs per core
KT = 16          # k tiles of 128
NQT = 4          # q chunks of 512
SCALE = 0.125    # 1/sqrt(64)
N_CORES = 8
LOOK = 3         # scores issued LOOK units ahead

_WAIT_CAP = {"InstEventSemaphore": 2}


def _split_multiwaits(nc):
    """This walrus build accepts 1 sync-wait per instruction (2 on
    EventSemaphore); spread extras over same-engine NOPs placed before."""
    n_spill = 0
    for f in nc.m.functions:
        for bb in f.blocks:
            new = []
            changed = False
            for inst in bb.instructions:
                si = inst.sync_info
                cap = _WAIT_CAP.get(type(inst).__name__, 1)
                if si is not None and len(si.on_wait) > cap:
                    extra = list(si.on_wait[: len(si.on_wait) - cap])
                    del si.on_wait[: len(si.on_wait) - cap]
                    for w in extra:
                        n_spill += 1
                        nop = mybir.InstNoOp(name=f"I-wspill-{n_spill}-{inst.name}")
                        nop.engine = inst.engine
                        nop.sync_info = mybir.SyncInfo(on_wait=[w], on_update=[])
                        new.append(nop)
                    changed = True
                new.append(inst)
            if changed:
                bb.instructions[:] = new
    return n_spill


def build_program():
    nc = bass.Bass("TRN2", target_bir_lowering=False, debug=False, num_devices=1)

    d_qt = nc.dram_tensor("qt", [DM, S], BF16, kind="ExternalInput").ap()
    d_kt = nc.dram_tensor("kt", [DM, S], BF16, kind="ExternalInput").ap()
    d_vt = nc.dram_tensor("vt", [DM, S], BF16, kind="ExternalInput").ap()
    d_wq = nc.dram_tensor("wq", [DM, F], BF16, kind="ExternalInput").ap()
    d_wk = nc.dram_tensor("wk", [DM, F], BF16, kind="ExternalInput").ap()
    d_wv = nc.dram_tensor("wv", [DM, F], BF16, kind="ExternalInput").ap()
    d_wo = nc.dram_tensor("wo", [F, DM], BF16, kind="ExternalInput").ap()
    d_bq = nc.dram_tensor("bq", [F], F32, kind="ExternalInput").ap()
    d_bk = nc.dram_tensor("bk", [F], F32, kind="ExternalInput").ap()
    d_bv = nc.dram_tensor("bv", [F], F32R, kind="ExternalInput").ap()
    d_ones = nc.dram_tensor("ones", [65, 128], F32R, kind="ExternalInput").ap()
    d_part = nc.dram_tensor("part", [S, DM], F32, kind="ExternalOutput").ap()

    with tile.TileContext(nc) as tc:
        with (
            tc.tile_pool(name="wpool", bufs=1) as wpool,
            tc.tile_pool(name="big", bufs=1) as big,
            tc.tile_pool(name="oTp", bufs=2) as oTp,
            tc.tile_pool(name="ktst", bufs=4) as ktst,
            tc.tile_pool(name="qtst", bufs=2) as qtst,
            tc.tile_pool(name="vtst", bufs=4) as vtst,
            tc.tile_pool(name="exch", bufs=4) as exch,
            tc.tile_pool(name="outst", bufs=2) as outst,
            tc.tile_pool(name="rcp", bufs=2) as rcp,
            tc.tile_pool(name="ocp", bufs=4) as ocp,
            tc.tile_pool(name="ps_sc", bufs=2, space="PSUM") as ps_sc,
            tc.tile_pool(name="ps_pv", bufs=2, space="PSUM") as ps_pv,
            tc.tile_pool(name="ps_acc", bufs=2, space="PSUM") as ps_acc,
        ):
            # ---- resident weight tiles (one merged DMA each)
            wq_sb = wpool.tile([128, 8 * F], BF16, tag="wq")
            wk_sb = wpool.tile([128, 8 * F], BF16, tag="wk")
            wv_sb = wpool.tile([128, 8 * F], BF16, tag="wv")
            wo_sb = wpool.tile([128, 4 * DM], BF16, tag="wo")
            bq_sb = wpool.tile([128, 4], F32, tag="bq")
            bk_sb = wpool.tile([128, 4], F32, tag="bk")
            bv_sb = wpool.tile([1, F], F32R, tag="bv")
            ones_sb = wpool.tile([65, 128], F32R, tag="ones")
            bvbc_sb = wpool.tile([128, F], F32, tag="bvbc")
            warm_sb = wpool.tile([1, 2], F32, tag="warm")
            dn2_sb = wpool.tile([65, 512], F32, tag="dn2")

            # big attention tiles: qT/kT hold the full token range per pair;
            # oT rotates per q-chunk (wo consumption finishes within 1 chunk)
            qT_sb = [big.tile([128, S], BF16, tag=f"qT{f}", name=f"qT{f}") for f in range(4)]
            kT_sb = [big.tile([128, S], BF16, tag=f"kT{f}", name=f"kT{f}") for f in range(4)]
            v_sb = [big.tile([128, NH * (HD + 1)], BF16, tag=f"v{t}", name=f"v{t}") for t in range(KT)]

            def qtr(msrc, b, n):
                return msrc[128 * b:128 * (b + 1), 512 * n:512 * (n + 1)]

            # ---- DMA issue, consumption-ordered, on the two usable DGE
            # queues.  sync: scores path (wk, kt); gpsimd: q/v path.
            nc.sync.dma_start(ones_sb[:], d_ones[:])
            nc.sync.dma_start(bq_sb[:], d_bq.rearrange("(f p) -> p f", p=128))
            nc.sync.dma_start(bk_sb[:], d_bk.rearrange("(f p) -> p f", p=128))
            nc.sync.dma_start(wk_sb[:].rearrange("p (b c) -> p b c", b=8), d_wk.rearrange("(b r) c -> r b c", r=128))
            nc.gpsimd.dma_start(bv_sb[:], d_bv.rearrange("(a f) -> a f", a=1))
            nc.gpsimd.dma_start(wq_sb[:].rearrange("p (b c) -> p b c", b=8), d_wq.rearrange("(b r) c -> r b c", r=128))
            kt_t, qt_t, vt_t = {}, {}, {}
            for b in range(8):
                t_ = ktst.tile([128, 512], BF16, tag=f"kt{b}", name=f"kt{b}n0")
                nc.sync.dma_start(t_[:], qtr(d_kt, b, 0))
                kt_t[(b, 0)] = t_
                t_ = qtst.tile([128, 512], BF16, tag=f"qt{b}", name=f"qt{b}n0")
                nc.gpsimd.dma_start(t_[:], qtr(d_qt, b, 0))
                qt_t[(b, 0)] = t_
            nc.gpsimd.dma_start(wv_sb[:].rearrange("p (b c) -> p b c", b=8), d_wv.rearrange("(b r) c -> r b c", r=128))
            for n in range(1, 4):
                for b in range(8):
                    t_ = ktst.tile([128, 512], BF16, tag=f"kt{b}", name=f"kt{b}n{n}")
                    nc.sync.dma_start(t_[:], qtr(d_kt, b, n))
                    kt_t[(b, n)] = t_
            for n in range(4):
                for b in range(8):
                    t_ = vtst.tile([128, 512], BF16, tag=f"vt{b}", name=f"vt{b}n{n}")
                    nc.gpsimd.dma_start(t_[:], qtr(d_vt, b, n))
                    vt_t[(b, n)] = t_
            nc.sync.dma_start(wo_sb[:].rearrange("p (b c) -> p b c", b=4), d_wo.rearrange("(b r) c -> r b c", r=128))
            for n in range(1, 4):
                for b in range(8):
                    t_ = qtst.tile([128, 512], BF16, tag=f"qt{b}", name=f"qt{b}n{n}")
                    nc.gpsimd.dma_start(t_[:], qtr(d_qt, b, n))
                    qt_t[(b, n)] = t_

            # preload the exp table set while DMA streams
            nc.scalar.activation(warm_sb[:], ones_sb[0:1, 0:2], EXP)

            # denominators of both heads live at partitions 0 and 64 of one
            # [65, 512] tile so a single reciprocal covers both (cost is
            # free-size bound); rows 1..63 are don't-care
            nc.vector.memset(dn2_sb[:], 1.0)

            # bv broadcast over partitions via K=1 matmul
            psbv = ps_acc.tile([128, 512], F32, tag="pacc", name="psbv")
            nc.tensor.matmul(psbv[:], ones_sb[0:1, :], bv_sb[0:1, :])
            nc.vector.tensor_copy(bvbc_sb[:], psbv[:])

            # ---- projection pieces: emitted as two 4-matmul halves --------
            def kq_halves(src_t, w_sb, bias_sb, dst_sb, n, f):
                st = {}

                def h1():
                    accp = ps_acc.tile([128, 512], F32, tag="pacc", name=f"acc{n}{f}")
                    st["acc"] = accp
                    for m in range(4):
                        nc.tensor.matmul(
                            accp[:],
                            w_sb[:, 512 * m + 128 * f:512 * m + 128 * (f + 1)],
                            src_t[(m, n)][:, :],
                            start=(m == 0), stop=False,
                        )

                def h2():
                    accp = st["acc"]
                    for m in range(4, 8):
                        nc.tensor.matmul(
                            accp[:],
                            w_sb[:, 512 * m + 128 * f:512 * m + 128 * (f + 1)],
                            src_t[(m, n)][:, :],
                            start=False, stop=(m == 7),
                        )
                    with nc.allow_low_precision(reason="bf16 qT/kT store"):
                        nc.vector.tensor_scalar_add(
                            dst_sb[f][:, 512 * n:512 * (n + 1)],
                            accp[:],
                            bias_sb[:, f:f + 1],
                        )
                return h1, h2

            def kq_piece(src_t, w_sb, bias_sb, dst_sb, n, f):
                h1, h2 = kq_halves(src_t, w_sb, bias_sb, dst_sb, n, f)
                h1()
                h2()

            def v_piece(t):
                q4, o = t // 4, 128 * (t % 4)
                acc = ps_acc.tile([128, 512], F32, tag="pacc", name=f"accv{t}")
                for m in range(8):
                    nc.tensor.matmul(
                        acc[:], vt_t[(m, q4)][:, o:o + 128],
                        wv_sb[:, 512 * m:512 * (m + 1)],
                        start=(m == 0), stop=(m == 7),
                    )
                v3 = v_sb[t][:].rearrange("p (h e) -> p h e", e=HD + 1)
                nc.vector.memset(v3[:, :, HD:HD + 1], 1.0)
                nc.vector.tensor_add(
                    v3[:, :, 0:HD],
                    acc[:].rearrange("p (h e) -> p h e", e=HD),
                    bvbc_sb[:].rearrange("p (h e) -> p h e", e=HD),
                )

            # ---- wo output projection pieces (chased) ------------------
            wo_pending = []
            oT_cur = [None] * 4
            wo_ost = {}
            wo_nq = [0]

            def emit_wo(count):
                # one call = one [128, 512] half; the merged [128, 1024] store
                # fires after the second half, alternating sync/gpsimd queues
                for _ in range(count):
                    if not wo_pending:
                        return
                    oTs, tt, j = wo_pending.pop(0)
                    pw = ps_acc.tile([128, 512], F32, tag="pacc", name="pw")
                    for f in range(4):
                        nc.tensor.matmul(
                            pw[:], oTs[f][:, 128 * (tt % 4):128 * (tt % 4 + 1)],
                            wo_sb[:, 1024 * f + 512 * j:1024 * f + 512 * (j + 1)],
                            start=(f == 0), stop=(f == 3),
                        )
                    if tt not in wo_ost:
                        wo_ost[tt] = outst.tile([128, 1024], F32, tag="outst", name=f"ost{tt}")
                    ost = wo_ost[tt]
                    nc.vector.tensor_copy(ost[:, 512 * j:512 * (j + 1)], pw[:])
                    if j == 1:
                        eng = nc.sync if wo_nq[0] % 2 == 0 else nc.gpsimd
                        wo_nq[0] += 1
                        eng.dma_start(
                            d_part[128 * tt:128 * (tt + 1), :], wo_ost.pop(tt)[:]
                        )

            # ---- per-group normalization, pipelined ---------------------
            def norm_p1(po, nm):
                oc = ocp.tile([65, 512], F32, tag="oc", name=nm)
                nc.vector.tensor_copy(oc[:], po[0:65, :])
                return oc

            def norm_recip(ocA, ocB):
                nc.vector.tensor_copy(dn2_sb[0:1, :], ocA[64:65, :])
                nc.vector.tensor_copy(dn2_sb[64:65, :], ocB[64:65, :])
                rc2 = rcp.tile([65, 512], F32R, tag="rc", name="rc2")
                with nc.allow_low_precision(reason="f32 reciprocal of denom"):
                    nc.vector.reciprocal(rc2[:], dn2_sb[:])
                return rc2

            def norm_bcast(rc2, i):
                # broadcast head i's reciprocal row to a base-0 [64, 512]
                pb = ps_acc.tile([128, 512], F32, tag="pacc", name=f"pb{i}")
                r = 64 * i
                nc.tensor.matmul(pb[0:64, :], ones_sb[r:r + 1, 0:64], rc2[r:r + 1, :])
                return pb

            def norm_mul(oc, pb, oTf, i):
                with nc.allow_low_precision(reason="bf16 normalized out"):
                    nc.vector.tensor_mul(
                        oTf[64 * i:64 * i + 64, :], oc[0:64, :], pb[0:64, :],
                    )

            # ---- minimal serial head: what group (0,0) m=0..3 needs
            for n4 in range(NQT):
                kq_piece(kt_t, wk_sb, bk_sb, kT_sb, n4, 0)
            kq_piece(qt_t, wq_sb, bq_sb, qT_sb, 0, 0)
            for t in range(4):
                v_piece(t)

            # ---- flat attention pipeline over all (group, m) units -----
            groups = [(n, p) for n in range(NQT) for p in range(NP)]

            def sc_emit(gi, m):
                n, p = groups[gi]
                qsl = slice(512 * n, 512 * (n + 1))
                scp = ps_sc.tile([128, 1024], F32, tag="sc")
                ksl = slice(128 * m, 128 * (m + 1))
                nc.tensor.matmul(
                    scp[:, 0:512], kT_sb[p][0:64, ksl], qT_sb[p][0:64, qsl],
                    tile_position=(0, 0),
                )
                nc.tensor.matmul(
                    scp[:, 512:1024], kT_sb[p][64:128, ksl],
                    qT_sb[p][64:128, qsl], tile_position=(64, 0),
                )
                ex = exch.tile([128, 1024], BF16, tag="ex")
                nc.scalar.activation(ex[:], scp[:], EXP, scale=SCALE)
                return ex

            # chase-slot tables.  kq pieces appear as (h1 slot, h2 slot);
            # deadlines account for the LOOK=3 score lookahead.
            def chase_slots(n, p):
                sl = {}

                def add(m, fn):
                    sl.setdefault(m, []).append(fn)

                if n == 0:
                    if p == 0:
                        h1, h2 = kq_halves(kt_t, wk_sb, bk_sb, kT_sb, 0, 1)
                        add(1, h1); add(2, h2)
                        for t in range(4, 16):
                            add(t - 1, lambda t=t: v_piece(t))
                        h1, h2 = kq_halves(kt_t, wk_sb, bk_sb, kT_sb, 1, 1)
                        add(5, h1); add(6, h2)
                        h1, h2 = kq_halves(qt_t, wq_sb, bq_sb, qT_sb, 0, 1)
                        add(10, h1); add(11, h2)
                    else:
                        h1, h2 = kq_halves(kt_t, wk_sb, bk_sb, kT_sb, 2, p)
                        add(0, h1); add(1, h2)
                        h1, h2 = kq_halves(kt_t, wk_sb, bk_sb, kT_sb, 3, p)
                        add(2, h1); add(3, h2)
                        if p < 3:
                            h1, h2 = kq_halves(kt_t, wk_sb, bk_sb, kT_sb, 0, p + 1)
                            add(4, h1); add(5, h2)
                            h1, h2 = kq_halves(kt_t, wk_sb, bk_sb, kT_sb, 1, p + 1)
                            add(6, h1); add(7, h2)
                            h1, h2 = kq_halves(qt_t, wq_sb, bq_sb, qT_sb, 0, p + 1)
                            add(10, h1); add(11, h2)
                if n + 1 < NQT:
                    h1, h2 = kq_halves(qt_t, wq_sb, bq_sb, qT_sb, n + 1, p)
                    add(10, h1) if (n == 0 and p == 0) else add(8, h1)
                    add(12, h2) if (n == 0 and p == 0) else add(9, h2)
                return sl

            NU = len(groups) * KT
            exq = {}
            for u0 in range(LOOK):
                exq[u0] = sc_emit(u0 // KT, u0 % KT)
            prev_state = None
            norm_st = {}

            for gi, (n, p) in enumerate(groups):
                if p == 0:
                    oT_cur = [oTp.tile([128, 512], BF16, tag=f"oT{f}", name=f"oT{f}c{n}")
                              for f in range(4)]
                oTs = oT_cur
                poA = ps_pv.tile([128, 512], F32, tag="po", name=f"poA{gi}")
                poB = ps_pv.tile([128, 512], F32, tag="po", name=f"poB{gi}")
                slots = chase_slots(n, p)
                for m in range(KT):
                    u = gi * KT + m
                    if u + LOOK < NU:
                        exq[u + LOOK] = sc_emit((u + LOOK) // KT, (u + LOOK) % KT)
                    ex = exq.pop(u)
                    nc.tensor.matmul(
                        poA[0:65, :], v_sb[m][:, 130 * p:130 * p + 65],
                        ex[:, 0:512], start=(m == 0), stop=(m == KT - 1),
                    )
                    nc.tensor.matmul(
                        poB[0:65, :], v_sb[m][:, 130 * p + 65:130 * p + 130],
                        ex[:, 512:1024], start=(m == 0), stop=(m == KT - 1),
                    )
                    # previous group's norm pipeline in this group's slack
                    if prev_state is not None:
                        pT, pp = prev_state
                        if m == 2:
                            norm_st["rc2"] = norm_recip(norm_st["ocA"], norm_st["ocB"])
                        elif m == 5:
                            norm_st["pbA"] = norm_bcast(norm_st["rc2"], 0)
                        elif m == 6:
                            norm_mul(norm_st["ocA"], norm_st["pbA"], pT[pp], 0)
                        elif m == 7:
                            norm_st["pbB"] = norm_bcast(norm_st["rc2"], 1)
                        elif m == 8:
                            norm_mul(norm_st["ocB"], norm_st["pbB"], pT[pp], 1)
                    for fn in slots.get(m, ()):
                        fn()
                    if m in (9, 12, 14):
                        emit_wo(1)
                # evacuate this group's PV psum right after the stop matmuls
                norm_st["ocA"] = norm_p1(poA, f"ocA{gi}")
                norm_st["ocB"] = norm_p1(poB, f"ocB{gi}")
                prev_state = (oTs, p)
                if p == NP - 1:
                    for t in range(4):
                        for j in range(2):
                            wo_pending.append((oTs, 4 * n + t, j))

            # ---- tail: last group's norms + remaining wo
            pT, pp = prev_state
            rc2 = norm_recip(norm_st["ocA"], norm_st["ocB"])
            pbA = norm_bcast(rc2, 0)
            norm_mul(norm_st["ocA"], pbA, pT[pp], 0)
            pbB = norm_bcast(rc2, 1)
            norm_mul(norm_st["ocB"], pbB, pT[pp], 1)
            emit_wo(len(wo_pending))

    _split_multiwaits(nc)
    return nc


_PROGRAM = None


def _get_program():
    global _PROGRAM
    if _PROGRAM is None:
        _PROGRAM = build_program()
    return _PROGRAM


def make_in_maps(Q, K, V, Wq, bq, Wk, bk, Wv, bv, Wo, bo):
    import ml_dtypes
    bf = lambda x: np.asarray(x, dtype=np.float32).astype(ml_dtypes.bfloat16)
    f32 = lambda x: np.asarray(x, dtype=np.float32)
    Q, K, V = bf(Q), bf(K), bf(V)
    Wq, Wk, Wv, Wo = bf(Wq), bf(Wk), bf(Wv), bf(Wo)
    bq, bk, bv = f32(bq), f32(bk), f32(bv)
    ones = np.ones((65, 128), np.float32)
    in_maps = []
    for c in range(N_CORES):
        b, hh = c // 2, c % 2
        fs = slice(F * hh, F * (hh + 1))
        in_maps.append({
            "qt": np.ascontiguousarray(Q[b].T),
            "kt": np.ascontiguousarray(K[b].T),
            "vt": np.ascontiguousarray(V[b].T),
            "wq": np.ascontiguousarray(Wq[:, fs]),
            "wk": np.ascontiguousarray(Wk[:, fs]),
            "wv": np.ascontiguousarray(Wv[:, fs]),
            "wo": np.ascontiguousarray(Wo[fs, :]),
            "bq": np.ascontiguousarray(bq[fs]),
            "bk": np.ascontiguousarray(bk[fs]),
            "bv": np.ascontiguousarray(bv[fs]),
            "ones": ones,
        })
    return in_maps


def kernel(Q, K, V, Wq, bq, Wk, bk, Wv, bv, Wo, bo, _trace=False, _trace_kwargs=None):
    nc = _get_program()
    in_maps = make_in_maps(Q, K, V, Wq, bq, Wk, bk, Wv, bv, Wo, bo)
    res = run_bass_kernel_spmd(
        nc, in_maps, core_ids=list(range(N_CORES)),
        trace=_trace, **(_trace_kwargs or {}),
    )
    parts = [r["part"] for r in res.results]
    out = np.stack([parts[2 * b] + parts[2 * b + 1] for b in range(B)])
    out += np.asarray(bo, dtype=np.float32)[None, None, :]
    if _trace:
        return out, res
    return out


# revision 16
# speedup vs baseline: 1.1903x; 1.0020x over previous
"""Multi-head attention (B=4, S=2048, D=1024, H=16) on 8 trn2 NeuronCores.

Sharding: (batch, head-half) -> one core each. Core c handles batch c//2 and
heads (c%2)*8 .. (c%2)*8+7 (feature columns (c%2)*512 .. +512 of the QKV
projections, rows of Wo). Each core computes its 8 heads' attention and a
partial output projection; the host sums the two partials per batch and adds
the output bias.

v4 schedule (vs the 540us baseline):
  - steady state is paced by the ACT engine (256 exps x ~1.05us); scores are
    issued 3 units ahead across group boundaries so ACT rides through PE
    bursts, and all chased projection work is split into 4-matmul halves
    spread over adjacent slots,
  - input DMA: quarter-granularity [128, 512] transfers in exact consumption
    order, split over the two usable DGE queues (sync HW + gpsimd SW; the
    scalar engine carries none, its FIFO would block ACTIVATEs),
  - softmax denominators of both heads are packed at partitions 0/64 of one
    [65, 512] tile -> a single DVE reciprocal per group,
  - wo partials merge into [128, 1024] stores alternating between queues.
Matmuls run in bf16 (fp32 PSUM accumulation); softmax denominators and
reciprocals stay fp32.
"""
import numpy as np

import concourse.bass as bass
import concourse.tile as tile
from concourse import mybir
from concourse.bass_utils import run_bass_kernel_spmd

F32 = mybir.dt.float32
F32R = mybir.dt.float32r
BF16 = mybir.dt.bfloat16
EXP = mybir.ActivationFunctionType.Exp

B, S, DM, H_TOT = 4, 2048, 1024, 16
F = 512          # features per core (8 heads x 64)
HD = 64          # head dim
NH = 8           # heads per core
NP = 4           # head pairs per core
KT = 16          # k tiles of 128
NQT = 4          # q chunks of 512
SCALE = 0.125    # 1/sqrt(64)
N_CORES = 8
LOOK = 3         # scores issued LOOK units ahead

_WAIT_CAP = {"InstEventSemaphore": 2}


def _split_multiwaits(nc):
    """This walrus build accepts 1 sync-wait per instruction (2 on
    EventSemaphore); spread extras over same-engine NOPs placed before."""
    n_spill = 0
    for f in nc.m.functions:
        for bb in f.blocks:
            new = []
            changed = False
            for inst in bb.instructions:
                si = inst.sync_info
                cap = _WAIT_CAP.get(type(inst).__name__, 1)
                if si is not None and len(si.on_wait) > cap:
                    extra = list(si.on_wait[: len(si.on_wait) - cap])
                    del si.on_wait[: len(si.on_wait) - cap]
                    for w in extra:
                        n_spill += 1
                        nop = mybir.InstNoOp(name=f"I-wspill-{n_spill}-{inst.name}")
                        nop.engine = inst.engine
                        nop.sync_info = mybir.SyncInfo(on_wait=[w], on_update=[])
                        new.append(nop)
                    changed = True
                new.append(inst)
            if changed:
                bb.instructions[:] = new
    return n_spill


def build_program():
    nc = bass.Bass("TRN2", target_bir_lowering=False, debug=False, num_devices=1)

    # super-layout inputs: row p of each [128, 4096] tensor holds the
    # per-128-block rows concatenated, so one DMA moves 1 MB with 8 KB
    # contiguous lines (341 GB/s vs ~90 GB/s for 0.125 MB transfers)
    d_qt = [nc.dram_tensor(f"qt{n}", [128, 4096], BF16, kind="ExternalInput").ap() for n in range(4)]
    d_kt = [nc.dram_tensor(f"kt{n}", [128, 4096], BF16, kind="ExternalInput").ap() for n in range(4)]
    d_vt = [nc.dram_tensor(f"vt{n}", [128, 4096], BF16, kind="ExternalInput").ap() for n in range(4)]
    d_wq = nc.dram_tensor("wq", [128, 4096], BF16, kind="ExternalInput").ap()
    d_wk = nc.dram_tensor("wk", [128, 4096], BF16, kind="ExternalInput").ap()
    d_wv = nc.dram_tensor("wv", [128, 4096], BF16, kind="ExternalInput").ap()
    d_wo = nc.dram_tensor("wo", [128, 4096], BF16, kind="ExternalInput").ap()
    d_bq = nc.dram_tensor("bq", [F], F32, kind="ExternalInput").ap()
    d_bk = nc.dram_tensor("bk", [F], F32, kind="ExternalInput").ap()
    d_bv = nc.dram_tensor("bv", [F], F32R, kind="ExternalInput").ap()
    d_ones = nc.dram_tensor("ones", [65, 128], F32R, kind="ExternalInput").ap()
    d_part = nc.dram_tensor("part", [S, DM], F32, kind="ExternalOutput").ap()

    with tile.TileContext(nc) as tc:
        with (
            tc.tile_pool(name="wpool", bufs=1) as wpool,
            tc.tile_pool(name="big", bufs=1) as big,
            tc.tile_pool(name="oTp", bufs=2) as oTp,
            tc.tile_pool(name="ktst", bufs=1) as ktst,
            tc.tile_pool(name="qtst", bufs=1) as qtst,
            tc.tile_pool(name="vtst", bufs=2) as vtst,
            tc.tile_pool(name="exch", bufs=4) as exch,
            tc.tile_pool(name="outst", bufs=2) as outst,
            tc.tile_pool(name="rcp", bufs=2) as rcp,
            tc.tile_pool(name="ocp", bufs=4) as ocp,
            tc.tile_pool(name="ps_sc", bufs=2, space="PSUM") as ps_sc,
            tc.tile_pool(name="ps_pv", bufs=2, space="PSUM") as ps_pv,
            tc.tile_pool(name="ps_acc", bufs=2, space="PSUM") as ps_acc,
        ):
            # ---- resident weight tiles (one merged DMA each)
            wq_sb = wpool.tile([128, 8 * F], BF16, tag="wq")
            wk_sb = wpool.tile([128, 8 * F], BF16, tag="wk")
            wv_sb = wpool.tile([128, 8 * F], BF16, tag="wv")
            wo_sb = wpool.tile([128, 4 * DM], BF16, tag="wo")
            bq_sb = wpool.tile([128, 4], F32, tag="bq")
            bk_sb = wpool.tile([128, 4], F32, tag="bk")
            bv_sb = wpool.tile([1, F], F32R, tag="bv")
            ones_sb = wpool.tile([65, 128], F32R, tag="ones")
            bvbc_sb = wpool.tile([128, F], F32, tag="bvbc")
            warm_sb = wpool.tile([1, 2], F32, tag="warm")
            dn2_sb = wpool.tile([65, 512], F32, tag="dn2")

            # big attention tiles: qT/kT hold the full token range per pair;
            # oT rotates per q-chunk (wo consumption finishes within 1 chunk)
            qT_sb = [big.tile([128, S], BF16, tag=f"qT{f}", name=f"qT{f}") for f in range(4)]
            kT_sb = [big.tile([128, S], BF16, tag=f"kT{f}", name=f"kT{f}") for f in range(4)]
            v_sb = [big.tile([128, NH * (HD + 1)], BF16, tag=f"v{t}", name=f"v{t}") for t in range(KT)]

            # ---- DMA: one 1 MB instruction per staged chunk, split over
            # the two usable DGE queues (sync HW + gpsimd SW; the scalar
            # engine carries none - its FIFO would block ACTIVATEs).
            nc.sync.dma_start(ones_sb[:], d_ones[:])
            nc.sync.dma_start(bq_sb[:], d_bq.rearrange("(f p) -> p f", p=128))
            nc.sync.dma_start(bk_sb[:], d_bk.rearrange("(f p) -> p f", p=128))
            nc.sync.dma_start(wk_sb[:], d_wk[:])
            nc.gpsimd.dma_start(bv_sb[:], d_bv.rearrange("(a f) -> a f", a=1))
            nc.gpsimd.dma_start(wq_sb[:], d_wq[:])
            kt_t, qt_t, vt_t = {}, {}, {}
            for n in range(4):
                t_ = ktst.tile([128, 4096], BF16, tag=f"kt{n}", name=f"kts{n}")
                nc.sync.dma_start(t_[:], d_kt[n][:])
                kt_t[n] = t_
            for n in range(2):
                t_ = qtst.tile([128, 4096], BF16, tag=f"qt{n}", name=f"qts{n}")
                nc.gpsimd.dma_start(t_[:], d_qt[n][:])
                qt_t[n] = t_
            nc.gpsimd.dma_start(wv_sb[:], d_wv[:])
            for n in range(4):
                t_ = vtst.tile([128, 4096], BF16, tag=f"vt{n % 2}", name=f"vts{n}")
                nc.gpsimd.dma_start(t_[:], d_vt[n][:])
                vt_t[n] = t_
            nc.sync.dma_start(wo_sb[:], d_wo[:])

            def load_late_qt():
                # late q chunks reuse the vt staging buffers; emitted only
                # after every v piece (the buffers' readers) exists, so the
                # WAR dependency is tracked before the overwrite
                for n in range(2, 4):
                    t_ = vtst.tile([128, 4096], BF16, tag=f"vt{n % 2}", name=f"qts{n}")
                    nc.sync.dma_start(t_[:], d_qt[n][:])
                    qt_t[n] = t_

            # preload the exp table set while DMA streams
            nc.scalar.activation(warm_sb[:], ones_sb[0:1, 0:2], EXP)

            # denominators of both heads live at partitions 0 and 64 of one
            # [65, 512] tile so a single reciprocal covers both (cost is
            # free-size bound); rows 1..63 are don't-care
            nc.vector.memset(dn2_sb[:], 1.0)

            # bv broadcast over partitions via K=1 matmul
            psbv = ps_acc.tile([128, 512], F32, tag="pacc", name="psbv")
            nc.tensor.matmul(psbv[:], ones_sb[0:1, :], bv_sb[0:1, :])
            nc.vector.tensor_copy(bvbc_sb[:], psbv[:])

            # ---- projection pieces: emitted as two 4-matmul halves --------
            def kq_halves(src_t, w_sb, bias_sb, dst_sb, n, f):
                st = {}

                def h1():
                    accp = ps_acc.tile([128, 512], F32, tag="pacc", name=f"acc{n}{f}")
                    st["acc"] = accp
                    for m in range(4):
                        nc.tensor.matmul(
                            accp[:],
                            w_sb[:, 512 * m + 128 * f:512 * m + 128 * (f + 1)],
                            src_t[n][:, 512 * m:512 * (m + 1)],
                            start=(m == 0), stop=False,
                        )

                def h2():
                    accp = st["acc"]
                    for m in range(4, 8):
                        nc.tensor.matmul(
                            accp[:],
                            w_sb[:, 512 * m + 128 * f:512 * m + 128 * (f + 1)],
                            src_t[n][:, 512 * m:512 * (m + 1)],
                            start=False, stop=(m == 7),
                        )
                    with nc.allow_low_precision(reason="bf16 qT/kT store"):
                        nc.vector.tensor_scalar_add(
                            dst_sb[f][:, 512 * n:512 * (n + 1)],
                            accp[:],
                            bias_sb[:, f:f + 1],
                        )
                return h1, h2

            def kq_piece(src_t, w_sb, bias_sb, dst_sb, n, f):
                h1, h2 = kq_halves(src_t, w_sb, bias_sb, dst_sb, n, f)
                h1()
                h2()

            def v_piece(t):
                q4, o = t // 4, 128 * (t % 4)
                acc = ps_acc.tile([128, 512], F32, tag="pacc", name=f"accv{t}")
                for m in range(8):
                    nc.tensor.matmul(
                        acc[:], vt_t[q4][:, 512 * m + o:512 * m + o + 128],
                        wv_sb[:, 512 * m:512 * (m + 1)],
                        start=(m == 0), stop=(m == 7),
                    )
                v3 = v_sb[t][:].rearrange("p (h e) -> p h e", e=HD + 1)
                nc.vector.memset(v3[:, :, HD:HD + 1], 1.0)
                nc.vector.tensor_add(
                    v3[:, :, 0:HD],
                    acc[:].rearrange("p (h e) -> p h e", e=HD),
                    bvbc_sb[:].rearrange("p (h e) -> p h e", e=HD),
                )

            # ---- wo output projection pieces (chased) ------------------
            wo_pending = []
            oT_cur = [None] * 4
            wo_ost = {}
            wo_nq = [0]

            def emit_wo(count):
                # one call = one [128, 512] half; the merged [128, 1024] store
                # fires after the second half, alternating sync/gpsimd queues
                for _ in range(count):
                    if not wo_pending:
                        return
                    oTs, tt, j = wo_pending.pop(0)
                    pw = ps_acc.tile([128, 512], F32, tag="pacc", name="pw")
                    for f in range(4):
                        nc.tensor.matmul(
                            pw[:], oTs[f][:, 128 * (tt % 4):128 * (tt % 4 + 1)],
                            wo_sb[:, 1024 * f + 512 * j:1024 * f + 512 * (j + 1)],
                            start=(f == 0), stop=(f == 3),
                        )
                    if tt not in wo_ost:
                        wo_ost[tt] = outst.tile([128, 1024], F32, tag="outst", name=f"ost{tt}")
                    ost = wo_ost[tt]
                    nc.vector.tensor_copy(ost[:, 512 * j:512 * (j + 1)], pw[:])
                    if j == 1:
                        eng = nc.sync if wo_nq[0] % 2 == 0 else nc.gpsimd
                        wo_nq[0] += 1
                        eng.dma_start(
                            d_part[128 * tt:128 * (tt + 1), :], wo_ost.pop(tt)[:]
                        )

            # ---- per-group normalization, pipelined ---------------------
            def norm_p1(po, nm):
                oc = ocp.tile([65, 512], F32, tag="oc", name=nm)
                nc.vector.tensor_copy(oc[:], po[0:65, :])
                return oc

            def norm_recip(ocA, ocB):
                nc.vector.tensor_copy(dn2_sb[0:1, :], ocA[64:65, :])
                nc.vector.tensor_copy(dn2_sb[64:65, :], ocB[64:65, :])
                rc2 = rcp.tile([65, 512], F32R, tag="rc", name="rc2")
                with nc.allow_low_precision(reason="f32 reciprocal of denom"):
                    nc.vector.reciprocal(rc2[:], dn2_sb[:])
                return rc2

            def norm_bcast(rc2, i):
                # broadcast head i's reciprocal row to a base-0 [64, 512]
                pb = ps_acc.tile([128, 512], F32, tag="pacc", name=f"pb{i}")
                r = 64 * i
                nc.tensor.matmul(pb[0:64, :], ones_sb[r:r + 1, 0:64], rc2[r:r + 1, :])
                return pb

            def norm_mul(oc, pb, oTf, i):
                with nc.allow_low_precision(reason="bf16 normalized out"):
                    nc.vector.tensor_mul(
                        oTf[64 * i:64 * i + 64, :], oc[0:64, :], pb[0:64, :],
                    )

            # ---- minimal serial head: what group (0,0) m=0..3 needs
            for n4 in range(NQT):
                kq_piece(kt_t, wk_sb, bk_sb, kT_sb, n4, 0)
            kq_piece(qt_t, wq_sb, bq_sb, qT_sb, 0, 0)
            for t in range(4):
                v_piece(t)

            # ---- flat attention pipeline over all (group, m) units -----
            groups = [(n, p) for n in range(NQT) for p in range(NP)]

            def sc_emit(gi, m):
                n, p = groups[gi]
                qsl = slice(512 * n, 512 * (n + 1))
                scp = ps_sc.tile([128, 1024], F32, tag="sc")
                ksl = slice(128 * m, 128 * (m + 1))
                nc.tensor.matmul(
                    scp[:, 0:512], kT_sb[p][0:64, ksl], qT_sb[p][0:64, qsl],
                    tile_position=(0, 0),
                )
                nc.tensor.matmul(
                    scp[:, 512:1024], kT_sb[p][64:128, ksl],
                    qT_sb[p][64:128, qsl], tile_position=(64, 0),
                )
                ex = exch.tile([128, 1024], BF16, tag="ex")
                nc.scalar.activation(ex[:], scp[:], EXP, scale=SCALE)
                return ex

            # chase-slot tables.  kq pieces appear as (h1 slot, h2 slot);
            # deadlines account for the LOOK=3 score lookahead.
            def chase_slots(n, p):
                sl = {}

                def add(m, fn):
                    sl.setdefault(m, []).append(fn)

                if n == 0:
                    if p == 0:
                        h1, h2 = kq_halves(kt_t, wk_sb, bk_sb, kT_sb, 0, 1)
                        add(1, h1); add(2, h2)
                        for t in range(4, 16):
                            add(t - 1, lambda t=t: v_piece(t))
                        h1, h2 = kq_halves(kt_t, wk_sb, bk_sb, kT_sb, 1, 1)
                        add(5, h1); add(6, h2)
                        h1, h2 = kq_halves(qt_t, wq_sb, bq_sb, qT_sb, 0, 1)
                        add(10, h1); add(11, h2)
                    else:
                        h1, h2 = kq_halves(kt_t, wk_sb, bk_sb, kT_sb, 2, p)
                        add(0, h1); add(1, h2)
                        h1, h2 = kq_halves(kt_t, wk_sb, bk_sb, kT_sb, 3, p)
                        add(2, h1); add(3, h2)
                        if p < 3:
                            h1, h2 = kq_halves(kt_t, wk_sb, bk_sb, kT_sb, 0, p + 1)
                            add(4, h1); add(5, h2)
                            h1, h2 = kq_halves(kt_t, wk_sb, bk_sb, kT_sb, 1, p + 1)
                            add(6, h1); add(7, h2)
                            h1, h2 = kq_halves(qt_t, wq_sb, bq_sb, qT_sb, 0, p + 1)
                            add(10, h1); add(11, h2)
                if n + 1 < NQT:
                    h1, h2 = kq_halves(qt_t, wq_sb, bq_sb, qT_sb, n + 1, p)
                    add(10, h1) if (n == 0 and p == 0) else add(8, h1)
                    add(12, h2) if (n == 0 and p == 0) else add(9, h2)
                return sl

            NU = len(groups) * KT
            exq = {}
            for u0 in range(LOOK):
                exq[u0] = sc_emit(u0 // KT, u0 % KT)
            prev_state = None
            norm_st = {}

            for gi, (n, p) in enumerate(groups):
                if gi == 1:
                    load_late_qt()
                if p == 0:
                    oT_cur = [oTp.tile([128, 512], BF16, tag=f"oT{f}", name=f"oT{f}c{n}")
                              for f in range(4)]
                oTs = oT_cur
                poA = ps_pv.tile([128, 512], F32, tag="po", name=f"poA{gi}")
                poB = ps_pv.tile([128, 512], F32, tag="po", name=f"poB{gi}")
                slots = chase_slots(n, p)
                for m in range(KT):
                    u = gi * KT + m
                    if u + LOOK < NU:
                        exq[u + LOOK] = sc_emit((u + LOOK) // KT, (u + LOOK) % KT)
                    ex = exq.pop(u)
                    nc.tensor.matmul(
                        poA[0:65, :], v_sb[m][:, 130 * p:130 * p + 65],
                        ex[:, 0:512], start=(m == 0), stop=(m == KT - 1),
                    )
                    nc.tensor.matmul(
                        poB[0:65, :], v_sb[m][:, 130 * p + 65:130 * p + 130],
                        ex[:, 512:1024], start=(m == 0), stop=(m == KT - 1),
                    )
                    # previous group's norm pipeline in this group's slack
                    if prev_state is not None:
                        pT, pp = prev_state
                        if m == 2:
                            norm_st["rc2"] = norm_recip(norm_st["ocA"], norm_st["ocB"])
                        elif m == 5:
                            norm_st["pbA"] = norm_bcast(norm_st["rc2"], 0)
                        elif m == 6:
                            norm_mul(norm_st["ocA"], norm_st["pbA"], pT[pp], 0)
                        elif m == 7:
                            norm_st["pbB"] = norm_bcast(norm_st["rc2"], 1)
                        elif m == 8:
                            norm_mul(norm_st["ocB"], norm_st["pbB"], pT[pp], 1)
                    for fn in slots.get(m, ()):
                        fn()
                    if m in (9, 10, 12, 13, 14):
                        emit_wo(1)
                # evacuate this group's PV psum right after the stop matmuls
                norm_st["ocA"] = norm_p1(poA, f"ocA{gi}")
                norm_st["ocB"] = norm_p1(poB, f"ocB{gi}")
                prev_state = (oTs, p)
                if p == NP - 1:
                    for t in range(4):
                        for j in range(2):
                            wo_pending.append((oTs, 4 * n + t, j))

            # ---- tail: last group's norms + remaining wo
            pT, pp = prev_state
            rc2 = norm_recip(norm_st["ocA"], norm_st["ocB"])
            pbA = norm_bcast(rc2, 0)
            norm_mul(norm_st["ocA"], pbA, pT[pp], 0)
            pbB = norm_bcast(rc2, 1)
            norm_mul(norm_st["ocB"], pbB, pT[pp], 1)
            emit_wo(len(wo_pending))

    _split_multiwaits(nc)
    return nc


_PROGRAM = None


def _get_program():
    global _PROGRAM
    if _PROGRAM is None:
        _PROGRAM = build_program()
    return _PROGRAM


def _sup(x2d):
    # [nb*128, C] -> [128, nb*C]: row p holds the 128-row blocks' p-th rows
    # concatenated, so a [128, C]-block view is x[:, C*m:C*(m+1)] and one
    # DMA moves the whole tensor with C-sized contiguous lines.
    nb, C = x2d.shape[0] // 128, x2d.shape[1]
    return np.ascontiguousarray(
        x2d.reshape(nb, 128, C).swapaxes(0, 1).reshape(128, nb * C))


def make_in_maps(Q, K, V, Wq, bq, Wk, bk, Wv, bv, Wo, bo):
    import ml_dtypes
    bf = lambda x: np.asarray(x, dtype=np.float32).astype(ml_dtypes.bfloat16)
    f32 = lambda x: np.asarray(x, dtype=np.float32)
    Q, K, V = bf(Q), bf(K), bf(V)
    Wq, Wk, Wv, Wo = bf(Wq), bf(Wk), bf(Wv), bf(Wo)
    bq, bk, bv = f32(bq), f32(bk), f32(bv)
    ones = np.ones((65, 128), np.float32)
    in_maps = []
    for c in range(N_CORES):
        b, hh = c // 2, c % 2
        fs = slice(F * hh, F * (hh + 1))
        qt, kt, vt = Q[b].T, K[b].T, V[b].T
        im = {
            "wq": _sup(Wq[:, fs]),
            "wk": _sup(Wk[:, fs]),
            "wv": _sup(Wv[:, fs]),
            "wo": _sup(Wo[fs, :]),
            "bq": np.ascontiguousarray(bq[fs]),
            "bk": np.ascontiguousarray(bk[fs]),
            "bv": np.ascontiguousarray(bv[fs]),
            "ones": ones,
        }
        for n in range(4):
            sl = slice(512 * n, 512 * (n + 1))
            im[f"qt{n}"] = _sup(qt[:, sl])
            im[f"kt{n}"] = _sup(kt[:, sl])
            im[f"vt{n}"] = _sup(vt[:, sl])
        in_maps.append(im)
    return in_maps


def kernel(Q, K, V, Wq, bq, Wk, bk, Wv, bv, Wo, bo, _trace=False, _trace_kwargs=None):
    nc = _get_program()
    in_maps = make_in_maps(Q, K, V, Wq, bq, Wk, bk, Wv, bv, Wo, bo)
    res = run_bass_kernel_spmd(
        nc, in_maps, core_ids=list(range(N_CORES)),
        trace=_trace, **(_trace_kwargs or {}),
    )
    parts = [r["part"] for r in res.results]
    out = np.stack([parts[2 * b] + parts[2 * b + 1] for b in range(B)])
    out += np.asarray(bo, dtype=np.float32)[None, None, :]
    if _trace:
        return out, res
    return out


# revision 18
# speedup vs baseline: 1.2529x; 1.0525x over previous
"""Multi-head attention (B=4, S=2048, D=1024, H=16) on 8 trn2 NeuronCores.

Sharding: (batch, head-half) -> one core each. Core c handles batch c//2 and
heads (c%2)*8 .. (c%2)*8+7 (feature columns (c%2)*512 .. +512 of the QKV
projections, rows of Wo). Each core computes its 8 heads' attention and a
partial output projection; the host sums the two partials per batch and adds
the output bias.

v4 schedule (vs the 540us baseline):
  - steady state is paced by the ACT engine (256 exps x ~1.05us); scores are
    issued 3 units ahead across group boundaries so ACT rides through PE
    bursts, and all chased projection work is split into 4-matmul halves
    spread over adjacent slots,
  - input DMA: quarter-granularity [128, 512] transfers in exact consumption
    order, split over the two usable DGE queues (sync HW + gpsimd SW; the
    scalar engine carries none, its FIFO would block ACTIVATEs),
  - softmax denominators of both heads are packed at partitions 0/64 of one
    [65, 512] tile -> a single DVE reciprocal per group,
  - wo partials merge into [128, 1024] stores alternating between queues.
Matmuls run in bf16 (fp32 PSUM accumulation); softmax denominators and
reciprocals stay fp32.
"""
import numpy as np

import concourse.bass as bass
import concourse.tile as tile
from concourse import mybir
from concourse.bass_utils import run_bass_kernel_spmd

F32 = mybir.dt.float32
F32R = mybir.dt.float32r
BF16 = mybir.dt.bfloat16
EXP = mybir.ActivationFunctionType.Exp

B, S, DM, H_TOT = 4, 2048, 1024, 16
F = 512          # features per core (8 heads x 64)
HD = 64          # head dim
NH = 8           # heads per core
NP = 4           # head pairs per core
KT = 16          # k tiles of 128
NQT = 4          # q chunks of 512
SCALE = 0.125    # 1/sqrt(64)
N_CORES = 8
LOOK = 3         # scores issued LOOK units ahead

_WAIT_CAP = {"InstEventSemaphore": 2}


def _split_multiwaits(nc):
    """This walrus build accepts 1 sync-wait per instruction (2 on
    EventSemaphore); spread extras over same-engine NOPs placed before."""
    n_spill = 0
    for f in nc.m.functions:
        for bb in f.blocks:
            new = []
            changed = False
            for inst in bb.instructions:
                si = inst.sync_info
                cap = _WAIT_CAP.get(type(inst).__name__, 1)
                if si is not None and len(si.on_wait) > cap:
                    extra = list(si.on_wait[: len(si.on_wait) - cap])
                    del si.on_wait[: len(si.on_wait) - cap]
                    for w in extra:
                        n_spill += 1
                        nop = mybir.InstNoOp(name=f"I-wspill-{n_spill}-{inst.name}")
                        nop.engine = inst.engine
                        nop.sync_info = mybir.SyncInfo(on_wait=[w], on_update=[])
                        new.append(nop)
                    changed = True
                new.append(inst)
            if changed:
                bb.instructions[:] = new
    return n_spill


def build_program():
    nc = bass.Bass("TRN2", target_bir_lowering=False, debug=False, num_devices=1)

    # super-layout inputs: row p of each [128, 4096] tensor holds the
    # per-128-block rows concatenated, so one DMA moves 1 MB with 8 KB
    # contiguous lines (341 GB/s vs ~90 GB/s for 0.125 MB transfers)
    d_qt = [nc.dram_tensor(f"qt{n}", [128, 4096], BF16, kind="ExternalInput").ap() for n in range(4)]
    d_kt = [nc.dram_tensor(f"kt{n}", [128, 4096], BF16, kind="ExternalInput").ap() for n in range(4)]
    d_vt = [nc.dram_tensor(f"vt{n}", [128, 4096], BF16, kind="ExternalInput").ap() for n in range(4)]
    d_wq = nc.dram_tensor("wq", [128, 4096], BF16, kind="ExternalInput").ap()
    d_wk = nc.dram_tensor("wk", [128, 4096], BF16, kind="ExternalInput").ap()
    d_wv = nc.dram_tensor("wv", [128, 4096], BF16, kind="ExternalInput").ap()
    d_wo = nc.dram_tensor("wo", [128, 4096], BF16, kind="ExternalInput").ap()
    d_bq = nc.dram_tensor("bq", [F], F32, kind="ExternalInput").ap()
    d_bk = nc.dram_tensor("bk", [F], F32, kind="ExternalInput").ap()
    d_bv = nc.dram_tensor("bv", [F], F32R, kind="ExternalInput").ap()
    d_ones = nc.dram_tensor("ones", [65, 128], F32R, kind="ExternalInput").ap()
    d_part = nc.dram_tensor("part", [S, DM], F32, kind="ExternalOutput").ap()

    with tile.TileContext(nc) as tc:
        with (
            tc.tile_pool(name="wpool", bufs=1) as wpool,
            tc.tile_pool(name="big", bufs=1) as big,
            tc.tile_pool(name="oTp", bufs=2) as oTp,
            tc.tile_pool(name="ktst", bufs=1) as ktst,
            tc.tile_pool(name="qtst", bufs=1) as qtst,
            tc.tile_pool(name="vtst", bufs=2) as vtst,
            tc.tile_pool(name="exch", bufs=4) as exch,
            tc.tile_pool(name="outst", bufs=2) as outst,
            tc.tile_pool(name="rcp", bufs=2) as rcp,
            tc.tile_pool(name="ocp", bufs=4) as ocp,
            tc.tile_pool(name="ps_sc", bufs=2, space="PSUM") as ps_sc,
            tc.tile_pool(name="ps_pv", bufs=2, space="PSUM") as ps_pv,
            tc.tile_pool(name="ps_acc", bufs=2, space="PSUM") as ps_acc,
        ):
            # ---- resident weight tiles (one merged DMA each)
            wq_sb = wpool.tile([128, 8 * F], BF16, tag="wq")
            wk_sb = wpool.tile([128, 8 * F], BF16, tag="wk")
            wv_sb = wpool.tile([128, 8 * F], BF16, tag="wv")
            wo_sb = wpool.tile([128, 4 * DM], BF16, tag="wo")
            bq_sb = wpool.tile([128, 4], F32, tag="bq")
            bk_sb = wpool.tile([128, 4], F32, tag="bk")
            bv_sb = wpool.tile([1, F], F32R, tag="bv")
            ones_sb = wpool.tile([65, 128], F32R, tag="ones")
            bvbc_sb = wpool.tile([128, F], F32, tag="bvbc")
            warm_sb = wpool.tile([1, 2], F32, tag="warm")
            dn2_sb = wpool.tile([65, 512], F32, tag="dn2")

            # big attention tiles: qT/kT hold the full token range per pair;
            # oT rotates per q-chunk (wo consumption finishes within 1 chunk)
            qT_sb = [big.tile([128, S], BF16, tag=f"qT{f}", name=f"qT{f}") for f in range(4)]
            kT_sb = [big.tile([128, S], BF16, tag=f"kT{f}", name=f"kT{f}") for f in range(4)]
            v_sb = [big.tile([128, NH * (HD + 1)], BF16, tag=f"v{t}", name=f"v{t}") for t in range(KT)]

            # ---- DMA: one 1 MB instruction per staged chunk, split over
            # the two usable DGE queues (sync HW + gpsimd SW; the scalar
            # engine carries none - its FIFO would block ACTIVATEs).
            nc.sync.dma_start(ones_sb[:], d_ones[:])
            nc.sync.dma_start(bq_sb[:], d_bq.rearrange("(f p) -> p f", p=128))
            nc.sync.dma_start(bk_sb[:], d_bk.rearrange("(f p) -> p f", p=128))
            nc.sync.dma_start(wk_sb[:], d_wk[:])
            nc.gpsimd.dma_start(bv_sb[:], d_bv.rearrange("(a f) -> a f", a=1))
            nc.gpsimd.dma_start(wq_sb[:], d_wq[:])
            kt_t, qt_t, vt_t = {}, {}, {}

            def gate(dst_t, src_t):
                # 1-elem gpsimd copy: dst's DMA (emitted next) WAW-waits this
                # copy, which RAW-waits src's transfer -> priority tiers that
                # defeat the SDMA round-robin across in-flight transfers
                nc.gpsimd.tensor_copy(dst_t[0:1, 0:1], src_t[0:1, 0:1])

            t_ = ktst.tile([128, 4096], BF16, tag="kt0", name="kts0")
            nc.sync.dma_start(t_[:], d_kt[0][:])
            kt_t[0] = t_
            t_ = qtst.tile([128, 4096], BF16, tag="qt0", name="qts0")
            nc.gpsimd.dma_start(t_[:], d_qt[0][:])
            qt_t[0] = t_
            for n in range(1, 4):
                t_ = ktst.tile([128, 4096], BF16, tag=f"kt{n}", name=f"kts{n}")
                gate(t_, kt_t[n - 1])
                nc.sync.dma_start(t_[:], d_kt[n][:])
                kt_t[n] = t_
                if n == 1:
                    nc.gpsimd.dma_start(wv_sb[:], d_wv[:])
                tv = vtst.tile([128, 4096], BF16, tag=f"vt{(n - 1) % 2}", name=f"vts{n - 1}")
                nc.gpsimd.dma_start(tv[:], d_vt[n - 1][:])
                vt_t[n - 1] = tv
            gate(wo_sb, kt_t[3])
            nc.sync.dma_start(wo_sb[:], d_wo[:])
            tv = vtst.tile([128, 4096], BF16, tag="vt1", name="vts3")
            nc.gpsimd.dma_start(tv[:], d_vt[3][:])
            vt_t[3] = tv
            t_ = qtst.tile([128, 4096], BF16, tag="qt1", name="qts1")
            gate(t_, vt_t[3])
            nc.gpsimd.dma_start(t_[:], d_qt[1][:])
            qt_t[1] = t_

            def load_late_qt():
                # late q chunks reuse the vt staging buffers; emitted only
                # after every v piece (the buffers' readers) exists, so the
                # WAR dependency is tracked before the overwrite
                for n in range(2, 4):
                    t_ = vtst.tile([128, 4096], BF16, tag=f"vt{n % 2}", name=f"qts{n}")
                    nc.sync.dma_start(t_[:], d_qt[n][:])
                    qt_t[n] = t_

            # preload the exp+ln table set while DMA streams
            nc.scalar.activation(warm_sb[:], ones_sb[0:1, 0:2], EXP)
            nc.scalar.activation(warm_sb[:], ones_sb[0:1, 0:2],
                                 mybir.ActivationFunctionType.Ln)

            # denominators of both heads live at partitions 0 and 64 of one
            # [65, 512] tile so a single reciprocal covers both (cost is
            # free-size bound); rows 1..63 are don't-care
            nc.vector.memset(dn2_sb[:], 1.0)

            # bv broadcast over partitions via K=1 matmul
            psbv = ps_acc.tile([128, 512], F32, tag="pacc", name="psbv")
            nc.tensor.matmul(psbv[:], ones_sb[0:1, :], bv_sb[0:1, :])
            nc.vector.tensor_copy(bvbc_sb[:], psbv[:])

            # ---- projection pieces: emitted as two 4-matmul halves --------
            def kq_halves(src_t, w_sb, bias_sb, dst_sb, n, f):
                st = {}

                def h1():
                    accp = ps_acc.tile([128, 512], F32, tag="pacc", name=f"acc{n}{f}")
                    st["acc"] = accp
                    for m in range(4):
                        nc.tensor.matmul(
                            accp[:],
                            w_sb[:, 512 * m + 128 * f:512 * m + 128 * (f + 1)],
                            src_t[n][:, 512 * m:512 * (m + 1)],
                            start=(m == 0), stop=False,
                        )

                def h2():
                    accp = st["acc"]
                    for m in range(4, 8):
                        nc.tensor.matmul(
                            accp[:],
                            w_sb[:, 512 * m + 128 * f:512 * m + 128 * (f + 1)],
                            src_t[n][:, 512 * m:512 * (m + 1)],
                            start=False, stop=(m == 7),
                        )
                    with nc.allow_low_precision(reason="bf16 qT/kT store"):
                        nc.vector.tensor_scalar_add(
                            dst_sb[f][:, 512 * n:512 * (n + 1)],
                            accp[:],
                            bias_sb[:, f:f + 1],
                        )
                return h1, h2

            def kq_piece(src_t, w_sb, bias_sb, dst_sb, n, f):
                h1, h2 = kq_halves(src_t, w_sb, bias_sb, dst_sb, n, f)
                h1()
                h2()

            def v_piece(t):
                q4, o = t // 4, 128 * (t % 4)
                acc = ps_acc.tile([128, 512], F32, tag="pacc", name=f"accv{t}")
                for m in range(8):
                    nc.tensor.matmul(
                        acc[:], vt_t[q4][:, 512 * m + o:512 * m + o + 128],
                        wv_sb[:, 512 * m:512 * (m + 1)],
                        start=(m == 0), stop=(m == 7),
                    )
                v3 = v_sb[t][:].rearrange("p (h e) -> p h e", e=HD + 1)
                nc.vector.memset(v3[:, :, HD:HD + 1], 1.0)
                nc.vector.tensor_add(
                    v3[:, :, 0:HD],
                    acc[:].rearrange("p (h e) -> p h e", e=HD),
                    bvbc_sb[:].rearrange("p (h e) -> p h e", e=HD),
                )

            # ---- wo output projection pieces (chased) ------------------
            wo_pending = []
            oT_cur = [None] * 4
            wo_ost = {}
            wo_nq = [0]

            def emit_wo(count):
                # one call = one [128, 512] half; the merged [128, 1024] store
                # fires after the second half, alternating sync/gpsimd queues
                for _ in range(count):
                    if not wo_pending:
                        return
                    oTs, tt, j = wo_pending.pop(0)
                    pw = ps_acc.tile([128, 512], F32, tag="pacc", name="pw")
                    for f in range(4):
                        nc.tensor.matmul(
                            pw[:], oTs[f][:, 128 * (tt % 4):128 * (tt % 4 + 1)],
                            wo_sb[:, 1024 * f + 512 * j:1024 * f + 512 * (j + 1)],
                            start=(f == 0), stop=(f == 3),
                        )
                    if tt not in wo_ost:
                        wo_ost[tt] = outst.tile([128, 1024], F32, tag="outst", name=f"ost{tt}")
                    ost = wo_ost[tt]
                    nc.vector.tensor_copy(ost[:, 512 * j:512 * (j + 1)], pw[:])
                    if j == 1:
                        eng = nc.sync if wo_nq[0] % 2 == 0 else nc.gpsimd
                        wo_nq[0] += 1
                        eng.dma_start(
                            d_part[128 * tt:128 * (tt + 1), :], wo_ost.pop(tt)[:]
                        )

            # ---- per-group normalization, pipelined ---------------------
            def norm_p1(po, nm):
                oc = ocp.tile([65, 512], F32, tag="oc", name=nm)
                nc.vector.tensor_copy(oc[:], po[0:65, :])
                return oc

            def norm_recip(ocA, ocB):
                # 1/d = exp(-ln d) on the ACT engine (same table set as the
                # softmax exp), freeing the DVE of 3.3us iterative divides
                nc.vector.tensor_copy(dn2_sb[0:1, :], ocA[64:65, :])
                nc.vector.tensor_copy(dn2_sb[64:65, :], ocB[64:65, :])
                ln2 = rcp.tile([65, 512], F32, tag="ln", name="ln2")
                rc2 = rcp.tile([65, 512], F32R, tag="rc", name="rc2")
                with nc.allow_low_precision(reason="recip via exp(-ln d)"):
                    nc.scalar.activation(ln2[:], dn2_sb[:],
                                         mybir.ActivationFunctionType.Ln)
                    nc.scalar.activation(rc2[:], ln2[:], EXP, scale=-1.0)
                return rc2

            def norm_bcast(rc2, i):
                # broadcast head i's reciprocal row to a base-0 [64, 512]
                pb = ps_acc.tile([128, 512], F32, tag="pacc", name=f"pb{i}")
                r = 64 * i
                nc.tensor.matmul(pb[0:64, :], ones_sb[r:r + 1, 0:64], rc2[r:r + 1, :])
                return pb

            def norm_mul(oc, pb, oTf, i):
                with nc.allow_low_precision(reason="bf16 normalized out"):
                    nc.vector.tensor_mul(
                        oTf[64 * i:64 * i + 64, :], oc[0:64, :], pb[0:64, :],
                    )

            # ---- minimal serial head: what group (0,0) m=0..3 needs
            for n4 in range(NQT):
                kq_piece(kt_t, wk_sb, bk_sb, kT_sb, n4, 0)
            kq_piece(qt_t, wq_sb, bq_sb, qT_sb, 0, 0)
            for t in range(4):
                v_piece(t)

            # ---- flat attention pipeline over all (group, m) units -----
            groups = [(n, p) for n in range(NQT) for p in range(NP)]

            def sc_emit(gi, m):
                n, p = groups[gi]
                qsl = slice(512 * n, 512 * (n + 1))
                scp = ps_sc.tile([128, 1024], F32, tag="sc")
                ksl = slice(128 * m, 128 * (m + 1))
                nc.tensor.matmul(
                    scp[:, 0:512], kT_sb[p][0:64, ksl], qT_sb[p][0:64, qsl],
                    tile_position=(0, 0),
                )
                nc.tensor.matmul(
                    scp[:, 512:1024], kT_sb[p][64:128, ksl],
                    qT_sb[p][64:128, qsl], tile_position=(64, 0),
                )
                ex = exch.tile([128, 1024], BF16, tag="ex")
                nc.scalar.activation(ex[:], scp[:], EXP, scale=SCALE)
                return ex

            # chase-slot tables.  kq pieces appear as (h1 slot, h2 slot);
            # deadlines account for the LOOK=3 score lookahead.
            def chase_slots(n, p):
                sl = {}

                def add(m, fn):
                    sl.setdefault(m, []).append(fn)

                if n == 0:
                    if p == 0:
                        h1, h2 = kq_halves(kt_t, wk_sb, bk_sb, kT_sb, 0, 1)
                        add(1, h1); add(2, h2)
                        for t in range(4, 16):
                            add(t - 1, lambda t=t: v_piece(t))
                        h1, h2 = kq_halves(kt_t, wk_sb, bk_sb, kT_sb, 1, 1)
                        add(5, h1); add(6, h2)
                        h1, h2 = kq_halves(qt_t, wq_sb, bq_sb, qT_sb, 0, 1)
                        add(10, h1); add(11, h2)
                    else:
                        h1, h2 = kq_halves(kt_t, wk_sb, bk_sb, kT_sb, 2, p)
                        add(0, h1); add(1, h2)
                        h1, h2 = kq_halves(kt_t, wk_sb, bk_sb, kT_sb, 3, p)
                        add(2, h1); add(3, h2)
                        if p < 3:
                            h1, h2 = kq_halves(kt_t, wk_sb, bk_sb, kT_sb, 0, p + 1)
                            add(4, h1); add(5, h2)
                            h1, h2 = kq_halves(kt_t, wk_sb, bk_sb, kT_sb, 1, p + 1)
                            add(6, h1); add(7, h2)
                            h1, h2 = kq_halves(qt_t, wq_sb, bq_sb, qT_sb, 0, p + 1)
                            add(10, h1); add(11, h2)
                if n + 1 < NQT:
                    h1, h2 = kq_halves(qt_t, wq_sb, bq_sb, qT_sb, n + 1, p)
                    add(10, h1) if (n == 0 and p == 0) else add(8, h1)
                    add(12, h2) if (n == 0 and p == 0) else add(9, h2)
                return sl

            NU = len(groups) * KT
            exq = {}
            for u0 in range(LOOK):
                exq[u0] = sc_emit(u0 // KT, u0 % KT)
            prev_state = None
            norm_st = {}

            for gi, (n, p) in enumerate(groups):
                if gi == 1:
                    load_late_qt()
                if p == 0:
                    oT_cur = [oTp.tile([128, 512], BF16, tag=f"oT{f}", name=f"oT{f}c{n}")
                              for f in range(4)]
                oTs = oT_cur
                poA = ps_pv.tile([128, 512], F32, tag="po", name=f"poA{gi}")
                poB = ps_pv.tile([128, 512], F32, tag="po", name=f"poB{gi}")
                slots = chase_slots(n, p)
                for m in range(KT):
                    u = gi * KT + m
                    if u + LOOK < NU:
                        exq[u + LOOK] = sc_emit((u + LOOK) // KT, (u + LOOK) % KT)
                    ex = exq.pop(u)
                    nc.tensor.matmul(
                        poA[0:65, :], v_sb[m][:, 130 * p:130 * p + 65],
                        ex[:, 0:512], start=(m == 0), stop=(m == KT - 1),
                    )
                    nc.tensor.matmul(
                        poB[0:65, :], v_sb[m][:, 130 * p + 65:130 * p + 130],
                        ex[:, 512:1024], start=(m == 0), stop=(m == KT - 1),
                    )
                    # previous group's norm pipeline in this group's slack
                    if prev_state is not None:
                        pT, pp = prev_state
                        if m == 2:
                            norm_st["rc2"] = norm_recip(norm_st["ocA"], norm_st["ocB"])
                        elif m == 5:
                            norm_st["pbA"] = norm_bcast(norm_st["rc2"], 0)
                        elif m == 6:
                            norm_mul(norm_st["ocA"], norm_st["pbA"], pT[pp], 0)
                        elif m == 7:
                            norm_st["pbB"] = norm_bcast(norm_st["rc2"], 1)
                        elif m == 8:
                            norm_mul(norm_st["ocB"], norm_st["pbB"], pT[pp], 1)
                    for fn in slots.get(m, ()):
                        fn()
                    if m in (9, 10, 12, 13, 14):
                        emit_wo(1)
                # evacuate this group's PV psum right after the stop matmuls
                norm_st["ocA"] = norm_p1(poA, f"ocA{gi}")
                norm_st["ocB"] = norm_p1(poB, f"ocB{gi}")
                prev_state = (oTs, p)
                if p == NP - 1:
                    for t in range(4):
                        for j in range(2):
                            wo_pending.append((oTs, 4 * n + t, j))

            # ---- tail: last group's norms + remaining wo
            pT, pp = prev_state
            rc2 = norm_recip(norm_st["ocA"], norm_st["ocB"])
            pbA = norm_bcast(rc2, 0)
            norm_mul(norm_st["ocA"], pbA, pT[pp], 0)
            pbB = norm_bcast(rc2, 1)
            norm_mul(norm_st["ocB"], pbB, pT[pp], 1)
            emit_wo(len(wo_pending))

    _split_multiwaits(nc)
    return nc


_PROGRAM = None


def _get_program():
    global _PROGRAM
    if _PROGRAM is None:
        _PROGRAM = build_program()
    return _PROGRAM


def _sup(x2d):
    # [nb*128, C] -> [128, nb*C]: row p holds the 128-row blocks' p-th rows
    # concatenated, so a [128, C]-block view is x[:, C*m:C*(m+1)] and one
    # DMA moves the whole tensor with C-sized contiguous lines.
    nb, C = x2d.shape[0] // 128, x2d.shape[1]
    return np.ascontiguousarray(
        x2d.reshape(nb, 128, C).swapaxes(0, 1).reshape(128, nb * C))


def make_in_maps(Q, K, V, Wq, bq, Wk, bk, Wv, bv, Wo, bo):
    import ml_dtypes
    bf = lambda x: np.asarray(x, dtype=np.float32).astype(ml_dtypes.bfloat16)
    f32 = lambda x: np.asarray(x, dtype=np.float32)
    Q, K, V = bf(Q), bf(K), bf(V)
    Wq, Wk, Wv, Wo = bf(Wq), bf(Wk), bf(Wv), bf(Wo)
    bq, bk, bv = f32(bq), f32(bk), f32(bv)
    ones = np.ones((65, 128), np.float32)
    in_maps = []
    for c in range(N_CORES):
        b, hh = c // 2, c % 2
        fs = slice(F * hh, F * (hh + 1))
        qt, kt, vt = Q[b].T, K[b].T, V[b].T
        im = {
            "wq": _sup(Wq[:, fs]),
            "wk": _sup(Wk[:, fs]),
            "wv": _sup(Wv[:, fs]),
            "wo": _sup(Wo[fs, :]),
            "bq": np.ascontiguousarray(bq[fs]),
            "bk": np.ascontiguousarray(bk[fs]),
            "bv": np.ascontiguousarray(bv[fs]),
            "ones": ones,
        }
        for n in range(4):
            sl = slice(512 * n, 512 * (n + 1))
            im[f"qt{n}"] = _sup(qt[:, sl])
            im[f"kt{n}"] = _sup(kt[:, sl])
            im[f"vt{n}"] = _sup(vt[:, sl])
        in_maps.append(im)
    return in_maps


def kernel(Q, K, V, Wq, bq, Wk, bk, Wv, bv, Wo, bo, _trace=False, _trace_kwargs=None):
    nc = _get_program()
    in_maps = make_in_maps(Q, K, V, Wq, bq, Wk, bk, Wv, bv, Wo, bo)
    res = run_bass_kernel_spmd(
        nc, in_maps, core_ids=list(range(N_CORES)),
        trace=_trace, **(_trace_kwargs or {}),
    )
    parts = [r["part"] for r in res.results]
    out = np.stack([parts[2 * b] + parts[2 * b + 1] for b in range(B)])
    out += np.asarray(bo, dtype=np.float32)[None, None, :]
    if _trace:
        return out, res
    return out
